# revision 1
# baseline (speedup 1.0000x reference)
"""Multi-head causal attention (B=2, S=4096, D=1024, H=16) on 8 TRN2 NeuronCores.

Sharding: head-parallel. Core c computes heads 2c, 2c+1 (128 of the 1024
projection columns) for both batches:
  - QKV column-parallel: each core gets Wq/Wk/Wv[:, c*128:(c+1)*128]
  - attention for its 2 heads over all tokens (causal, flash-free: full
    score rows, streamed in 512-query chunks, keys on PSUM partitions;
    the two heads' score matmuls are packed into PE row-groups 0-63 /
    64-127 via tile_position and run concurrently)
  - out-proj row-parallel: partial_out = ctx_c @ Wo[c*128:(c+1)*128, :]
  - host sums the 8 partials and adds bo.

x is transposed on the host (xT = x.reshape(T, D).T) because every matmul
on the PE contracts over the partition axis; this avoids all on-chip
transposes.

Layouts on-chip (per core):
  qT, kT:  [128, T]  rows 0:64 head0, 64:128 head1 (transposed projections)
  vA:      [128, T/128, 130]  per key-tile [v_h0 | ones | v_h1 | ones]
           (the ones column makes the ctx matmul also produce the softmax
           denominator as output row 64)
  scoresT: PSUM [128 keys, 2x512 queries (both heads)] -> exp on ACT -> SBUF
  ctxT:    [128, T]  accumulated in PSUM [65, 512] per (head, query chunk);
           softmax denominators ride along as output row 64
"""

from contextlib import ExitStack

import numpy as np

import concourse.bass as bass
import concourse.tile as tile
from concourse import bacc, mybir
from concourse.bass_utils import run_bass_kernel_spmd

F32 = mybir.dt.float32
BF16 = mybir.dt.bfloat16
P = 128
AF = mybir.ActivationFunctionType

N_CORES = 8
B_FULL, S_FULL, D_FULL, H_FULL = 2, 4096, 1024, 16
DH = 64
CW = 128  # projection columns per core (2 heads * 64)


def build_program(S=S_FULL, B=B_FULL, D=D_FULL):
    """Build the per-core Bass program (same program on all 8 cores)."""
    T = B * S
    KC = D // P            # contraction chunks for the projections
    IC = min(512, S)       # query-chunk width (paired-head layout)
    NJ = S // P            # key tiles per batch
    NIC = S // IC          # query chunks per batch
    WN = min(512, T)       # QKV token window

    nc = bacc.Bacc("TRN2", target_bir_lowering=False, debug=False,
                   num_devices=N_CORES)

    xT = nc.dram_tensor("xT", [D, T], BF16, kind="ExternalInput").ap()
    wq = nc.dram_tensor("wq", [P, D // P, CW], BF16, kind="ExternalInput").ap()
    wk = nc.dram_tensor("wk", [P, D // P, CW], BF16, kind="ExternalInput").ap()
    wv = nc.dram_tensor("wv", [P, D // P, CW], BF16, kind="ExternalInput").ap()
    wo = nc.dram_tensor("wo", [CW, D], BF16, kind="ExternalInput").ap()
    mask = nc.dram_tensor("mask", [P, P], BF16, kind="ExternalInput").ap()
    out = nc.dram_tensor("out", [T, D], BF16, kind="ExternalOutput").ap()
    # bounce rows for broadcasting 1/sum across partitions via DMA
    nrm = nc.dram_tensor("nrm_scratch", [B * NIC * 2, IC], F32).ap()

    with tile.TileContext(nc) as tc, ExitStack() as ctx:
        singles = ctx.enter_context(tc.tile_pool(name="singles", bufs=1))
        qT = singles.tile([P, T], BF16, name="qT")
        kT = singles.tile([P, T], BF16, name="kT")
        vA = singles.tile([P, B * NJ, 130], BF16, name="vA")
        cT = singles.tile([P, T], BF16, name="cT")
        wq_s = singles.tile([P, KC, CW], BF16, name="wq_s")
        wk_s = singles.tile([P, KC, CW], BF16, name="wk_s")
        wv_s = singles.tile([P, KC, CW], BF16, name="wv_s")
        wo_s = singles.tile([CW, D], BF16, name="wo_s")
        mask_s = singles.tile([P, P], BF16, name="mask_s")
        ones65 = singles.tile([65, 64], F32, name="ones65")

        nc.sync.dma_start(out=wq_s, in_=wq)
        nc.sync.dma_start(out=wk_s, in_=wk)
        nc.sync.dma_start(out=wv_s, in_=wv)
        nc.vector.memset(ones65[64:65, :], 1.0)
        nc.vector.memset(vA[:, :, 64:65], 1.0)
        nc.vector.memset(vA[:, :, 129:130], 1.0)

        xw_pool = ctx.enter_context(tc.tile_pool(name="xw_pool", bufs=3))
        # PSUM budget (8 banks): sc 2x2 + cx 2x1 + shared-small 2x1 = 8
        sm_ps = ctx.enter_context(
            tc.tile_pool(name="sm_ps", bufs=2, space=bass.MemorySpace.PSUM))
        sc_ps = ctx.enter_context(
            tc.tile_pool(name="sc_ps", bufs=2, space=bass.MemorySpace.PSUM))
        cx_ps = ctx.enter_context(
            tc.tile_pool(name="cx_ps", bufs=2, space=bass.MemorySpace.PSUM))
        exp_sb = ctx.enter_context(tc.tile_pool(name="exp_sb", bufs=4))
        st_sb = ctx.enter_context(tc.tile_pool(name="st_sb", bufs=2))
        nrm_sb = ctx.enter_context(tc.tile_pool(name="nrm_sb", bufs=2))
        ob_sb = ctx.enter_context(tc.tile_pool(name="ob_sb", bufs=2))

        def emit_qkv_window(w):
            xw = xw_pool.tile([P, KC, WN], BF16, name="xw", tag="xw")
            for kc in range(KC):
                nc.sync.dma_start(
                    out=xw[:, kc, :],
                    in_=xT[kc * P:(kc + 1) * P, w * WN:(w + 1) * WN])
            q_ps = sm_ps.tile([P, WN], F32, name="q_ps", tag="sm")
            for kc in range(KC):
                nc.tensor.matmul(q_ps, wq_s[:, kc, :], xw[:, kc, :],
                                 start=(kc == 0), stop=(kc == KC - 1))
            nc.vector.tensor_copy(qT[:, w * WN:(w + 1) * WN], q_ps)
            k_ps = sm_ps.tile([P, WN], F32, name="k_ps", tag="sm")
            for kc in range(KC):
                nc.tensor.matmul(k_ps, wk_s[:, kc, :], xw[:, kc, :],
                                 start=(kc == 0), stop=(kc == KC - 1))
            nc.vector.tensor_copy(kT[:, w * WN:(w + 1) * WN], k_ps)
            for st in range(WN // P):
                jt = (w * WN) // P + st  # global token tile
                vp = sm_ps.tile([P, CW], F32, name="vp", tag="sm")
                for kc in range(KC):
                    nc.tensor.matmul(vp, xw[:, kc, st * P:(st + 1) * P],
                                     wv_s[:, kc, :],
                                     start=(kc == 0), stop=(kc == KC - 1))
                nc.vector.tensor_copy(vA[:, jt, 0:64], vp[:, 0:64])
                nc.vector.tensor_copy(vA[:, jt, 65:129], vp[:, 64:128])

        def emit_attn_chunk(b, icn, tail=False):
            gi0 = b * S + icn * IC  # global query start
            njt = (icn + 1) * (IC // P)
            # one cx tile [65, IC<=512] (1 bank) per head
            cxs = [cx_ps.tile([65, IC], F32, name="cx", tag="cx")
                   for _ in range(2)]
            seen = [set(), set()]

            def emit_ctx(jt, ex, cpieces):
                for (h, a, bnd, stp) in cpieces:
                    strt = jt == 0 and 0 not in seen[h]
                    seen[h].add(0)
                    nc.tensor.matmul(
                        cxs[h][:, a:bnd],
                        vA[:, b * NJ + jt, h * 65:(h + 1) * 65],
                        ex[:, h * IC + a:h * IC + bnd],
                        start=strt, stop=stp)

            pend = None
            for jt in range(njt):
                il0 = max(0, jt * P - icn * IC)
                gj0 = b * S + jt * P
                # paired scores: h0 -> cols [0:IC), h1 -> cols [IC:2IC)
                # of one psum tile; tile_position row-split (0,0)/(64,0)
                # lets the two matmuls run concurrently on the PE
                sc = sc_ps.tile([P, 2 * IC], F32, name="sc", tag="sc")
                for h in range(2):
                    hp = h * 64
                    nc.tensor.matmul(
                        sc[:, h * IC + il0:(h + 1) * IC],
                        kT[hp:hp + 64, gj0:gj0 + P],
                        qT[hp:hp + 64, gi0 + il0:gi0 + IC],
                        start=True, stop=True)
                ex = exp_sb.tile([P, 2 * IC], BF16, name="ex", tag="ex")
                if il0 == 0:
                    nc.scalar.activation(ex[:, 0:2 * IC], sc[:, 0:2 * IC],
                                         AF.Exp, scale=0.125)
                else:
                    # diagonal tiles: the scores matmuls only wrote
                    # [il0, IC) per head, so exp each head's range
                    for h in range(2):
                        nc.scalar.activation(
                            ex[:, h * IC + il0:(h + 1) * IC],
                            sc[:, h * IC + il0:(h + 1) * IC],
                            AF.Exp, scale=0.125)
                diag = jt * P >= icn * IC
                if diag:  # mask both heads' diagonal blocks
                    nc.vector.tensor_mul(ex[:, il0:il0 + P],
                                         ex[:, il0:il0 + P], mask_s)
                    nc.vector.tensor_mul(ex[:, IC + il0:IC + il0 + P],
                                         ex[:, IC + il0:IC + il0 + P],
                                         mask_s)
                # ctx pieces per head; stop on the final diagonal piece
                cpieces = []
                for h in range(2):
                    if diag:
                        cpieces.append((h, il0, il0 + P, il0 + P == IC))
                        if il0 + P < IC:
                            cpieces.append((h, il0 + P, IC, False))
                    else:
                        cpieces.append((h, il0, IC, False))
                # lag-1 software pipeline
                if pend is not None:
                    emit_ctx(*pend)
                pend = (jt, ex, cpieces)
            emit_ctx(*pend)
            # evacuate + normalize both heads (h1 first: its
            # chain has an extra DMA; overlap it under h0's)
            for h in (1, 0):
                stage = st_sb.tile([65, IC], F32, name="stage", tag="stage")
                nc.vector.tensor_copy(stage, cxs[h])
                nc.vector.reciprocal(stage[64:65, :], stage[64:65, :])
                if tail:
                    # PE is idle at the kernel tail: broadcast 1/sum
                    # across partitions with a K=1 matmul instead of the
                    # higher-latency DRAM-bounce DMA pair
                    rb = sm_ps.tile([64, IC], F32, name="rbp", tag="sm")
                    nc.tensor.matmul(rb, ones65[64:65, :],
                                     stage[64:65, :], start=True, stop=True)
                else:
                    ni = (b * NIC + icn) * 2 + h
                    nc.sync.dma_start(out=nrm[ni:ni + 1, :],
                                      in_=stage[64:65, :])
                    src = nrm[ni:ni + 1, :]
                    bc = bass.AP(tensor=src.tensor, offset=src.offset,
                                 ap=[[0, 64], src.ap[-1]])
                    rb = nrm_sb.tile([64, IC], F32, name="rb", tag="rb")
                    nc.sync.dma_start(out=rb, in_=bc)
                if h == 0:
                    nc.vector.tensor_mul(cT[0:64, gi0:gi0 + IC],
                                         stage[0:64, :], rb)
                else:
                    tmp = nrm_sb.tile([64, IC], BF16, name="tmp", tag="tmp")
                    nc.vector.tensor_mul(tmp, stage[0:64, :], rb)
                    nc.sync.dma_start(out=cT[64:128, gi0:gi0 + IC], in_=tmp)
            # out-projection for this query chunk
            for st in range(IC // P):
                s0 = gi0 + st * P
                ob = ob_sb.tile([P, D], BF16, name="ob", tag="ob")
                for nn in range(D // 512):
                    op = sm_ps.tile([P, 512], F32, name="op", tag="sm")
                    nc.tensor.matmul(op, cT[:, s0:s0 + P],
                                     wo_s[:, nn * 512:(nn + 1) * 512],
                                     start=True, stop=True)
                    if tail and nn % 2 == 0:
                        # ACT is idle at the kernel tail; split the psum
                        # evacuation across both engines
                        nc.scalar.copy(ob[:, nn * 512:(nn + 1) * 512], op)
                    else:
                        nc.vector.tensor_copy(ob[:, nn * 512:(nn + 1) * 512],
                                              op)
                nc.sync.dma_start(out=out[s0:s0 + P, :], in_=ob)

        # Emission: fully pipelined. Window w covers tokens
        # [w*WN,(w+1)*WN); chunk (b, icn) only needs windows covering
        # tokens < b*S + (icn+1)*IC. Emitting windows two ahead of the
        # chunk that needs them keeps their DMA off the critical path
        # while attention (ACT-bound) overlaps the projection matmuls.
        nwin = T // WN
        state = {"emitted": 0}

        def need(upto):
            while state["emitted"] < min(upto, nwin):
                emit_qkv_window(state["emitted"])
                if state["emitted"] == 0:
                    # deferred: not needed before the first attention chunk
                    nc.sync.dma_start(out=wo_s, in_=wo)
                    nc.sync.dma_start(out=mask_s, in_=mask)
                state["emitted"] += 1

        for b in range(B):
            for icn in range(NIC):
                need((b * S + (icn + 1) * IC + WN - 1) // WN + 2)
                emit_attn_chunk(b, icn,
                                tail=(b == B - 1 and icn == NIC - 1))
        need(nwin)

    nc.compile()
    return nc


def _warrange(w, bf16):
    # [D, CW] -> [P, D//P, CW] contiguous (the SBUF layout, so the DMA is
    # a single contiguous copy instead of 256B strided pieces)
    D, CW_ = w.shape
    return np.ascontiguousarray(
        w.reshape(D // P, P, CW_).transpose(1, 0, 2)).astype(bf16)


def make_in_maps(x, Wq, Wk, Wv, Wo):
    import ml_dtypes
    bf16 = ml_dtypes.bfloat16
    B, S, D = x.shape
    xT = np.ascontiguousarray(x.reshape(B * S, D).T).astype(bf16)
    mask = np.triu(np.ones((P, P), dtype=bf16))
    in_maps = []
    for c in range(N_CORES):
        cs = slice(c * CW, (c + 1) * CW)
        in_maps.append({
            "xT": xT,
            "wq": _warrange(Wq[:, cs], bf16),
            "wk": _warrange(Wk[:, cs], bf16),
            "wv": _warrange(Wv[:, cs], bf16),
            "wo": np.ascontiguousarray(Wo[cs, :]).astype(bf16),
            "mask": mask,
        })
    return in_maps


_CACHED_NC = None


def kernel(x, Wq, Wk, Wv, Wo, bo, _trace=False):
    global _CACHED_NC
    x = np.asarray(x, dtype=np.float32)
    B, S, D = x.shape
    if _CACHED_NC is None:
        _CACHED_NC = build_program(S=S, B=B, D=D)
    nc = _CACHED_NC
    in_maps = make_in_maps(x, np.asarray(Wq), np.asarray(Wk),
                           np.asarray(Wv), np.asarray(Wo))
    res = None
    for attempt in range(3):
        try:
            res = run_bass_kernel_spmd(nc, in_maps, list(range(N_CORES)),
                                       trace=_trace)
            break
        except Exception:
            if attempt == 2:
                raise
    out = np.zeros((B * S, D), dtype=np.float32)
    for c in range(N_CORES):
        out += res.results[c]["out"].astype(np.float32)
    out += np.asarray(bo, dtype=np.float32)[None, :]
    if _trace:
        kernel._last_result = res
    return out.reshape(B, S, D)



# revision 2
# speedup vs baseline: 1.1745x; 1.1745x over previous
"""Multi-head causal attention (B=2, S=4096, D=1024, H=16) on 8 TRN2 NeuronCores.

Sharding: head-parallel. Core c computes heads 2c, 2c+1 (128 of the 1024
projection columns) for both batches:
  - QKV column-parallel: each core gets Wq/Wk/Wv[:, c*128:(c+1)*128]
  - out-proj row-parallel: partial_out = ctx_c @ Wo[c*128:(c+1)*128, :]
  - host sums the 8 partials and adds bo.

The TimelineSim cost model charges a matmul only by its OUTPUT free size
(N columns), so the ctx product is blocked transposed: per (128-query x
128-key) causal block, out[128 q, 65] += ex[128 k, 128 q].T @ vA[128 k, 65]
streams just 65 columns (the 65th is an all-ones column producing the
softmax denominator), instead of re-streaming 512 query columns per key
tile. The denominator then sits on the free dim, so normalization is a
cheap per-partition tensor_scalar, and a single PE transpose per query
tile restores the [cdim, token] layout for the out-projection.

Layouts on-chip (per core):
  qT, kT:  [128, T]  rows 0:64 head0, 64:128 head1 (transposed projections)
  vA:      [128, T/128, 130]  per key-tile [v_h0 | ones | v_h1 | ones]
  sc/ex:   [128 keys, 2 heads, 512 queries]  PSUM scores / SBUF exp(bf16)
  acc:     PSUM [128 q, 2 qsub, 2 heads, 65] x2 (unnormalized ctx + denom)
  cT:      [128 cdim, T]  normalized ctx, transposed back per 128-q tile

Emission keeps PE uniformly loaded: QKV projection windows and per-tile
tail work (normalize/transpose/out-proj) are closures on a work queue,
pumped one or two per attention iteration.
"""

from collections import deque
from contextlib import ExitStack

import numpy as np

import concourse.bass as bass
import concourse.tile as tile
from concourse import bacc, mybir
from concourse.bass_utils import run_bass_kernel_spmd

F32 = mybir.dt.float32
BF16 = mybir.dt.bfloat16
P = 128
AF = mybir.ActivationFunctionType

N_CORES = 8
B_FULL, S_FULL, D_FULL, H_FULL = 2, 4096, 1024, 16
DH = 64
CW = 128  # projection columns per core (2 heads * 64)


def build_program(S=S_FULL, B=B_FULL, D=D_FULL):
    """Build the per-core Bass program (same program on all 8 cores)."""
    T = B * S
    KC = D // P            # contraction chunks for the projections
    IC = min(512, S)       # query-chunk width
    QT = IC // P           # query tiles per chunk (4)
    NJ = S // P            # key tiles per batch
    NIC = S // IC          # query chunks per batch
    WN = min(512, T)       # QKV token window

    nc = bacc.Bacc("TRN2", target_bir_lowering=False, debug=False,
                   num_devices=N_CORES)

    xT = nc.dram_tensor("xT", [D, T], BF16, kind="ExternalInput").ap()
    wq = nc.dram_tensor("wq", [P, D // P, CW], BF16, kind="ExternalInput").ap()
    wk = nc.dram_tensor("wk", [P, D // P, CW], BF16, kind="ExternalInput").ap()
    wv = nc.dram_tensor("wv", [P, D // P, CW], BF16, kind="ExternalInput").ap()
    wo = nc.dram_tensor("wo", [CW, D], BF16, kind="ExternalInput").ap()
    mask = nc.dram_tensor("mask", [P, P], BF16, kind="ExternalInput").ap()
    ident = nc.dram_tensor("ident", [P, P], BF16, kind="ExternalInput").ap()
    out = nc.dram_tensor("out", [T, D], BF16, kind="ExternalOutput").ap()

    with tile.TileContext(nc) as tc, ExitStack() as ctx:
        singles = ctx.enter_context(tc.tile_pool(name="singles", bufs=1))
        qT = singles.tile([P, T], BF16, name="qT")
        kT = singles.tile([P, T], BF16, name="kT")
        vA = singles.tile([P, B * NJ, 130], BF16, name="vA")
        cT = singles.tile([P, T], BF16, name="cT")
        wq_s = singles.tile([P, KC, CW], BF16, name="wq_s")
        wk_s = singles.tile([P, KC, CW], BF16, name="wk_s")
        wv_s = singles.tile([P, KC, CW], BF16, name="wv_s")
        wo_s = singles.tile([CW, D], BF16, name="wo_s")
        mask_s = singles.tile([P, P], BF16, name="mask_s")
        ident_s = singles.tile([P, P], BF16, name="ident_s")

        nc.sync.dma_start(out=wq_s, in_=wq)
        nc.sync.dma_start(out=wk_s, in_=wk)
        nc.sync.dma_start(out=wv_s, in_=wv)
        nc.vector.memset(vA[:, :, 64:65], 1.0)
        nc.vector.memset(vA[:, :, 129:130], 1.0)
        warm = singles.tile([P, 512], BF16, name="warm")
        nc.vector.memset(warm, 0.0)

        # PSUM budget (8 banks): sc 2x2 + acc 2x1 + shared-small 2x1 = 8
        ps_single = ctx.enter_context(
            tc.tile_pool(name="ps_single", bufs=1, space=bass.MemorySpace.PSUM))
        # [q, qsub-pair, head, v+denom]; accA holds qsub 0,1; accB 2,3
        accA = ps_single.tile([P, 2, 2, 65], F32, name="accA")
        accB = ps_single.tile([P, 2, 2, 65], F32, name="accB")
        sc_ps = ctx.enter_context(
            tc.tile_pool(name="sc_ps", bufs=2, space=bass.MemorySpace.PSUM))
        sm_ps = ctx.enter_context(
            tc.tile_pool(name="sm_ps", bufs=2, space=bass.MemorySpace.PSUM))
        xw_pool = ctx.enter_context(tc.tile_pool(name="xw_pool", bufs=10))
        ex_sb = ctx.enter_context(tc.tile_pool(name="ex_sb", bufs=4))
        nrm_sb = ctx.enter_context(tc.tile_pool(name="nrm_sb", bufs=2))
        rcp_sb = ctx.enter_context(tc.tile_pool(name="rcp_sb", bufs=2))
        ob_sb = ctx.enter_context(tc.tile_pool(name="ob_sb", bufs=2))

        # ---- deferred-work queue of generators yielding ~PE-ns steps ----
        # Filler (QKV windows) is metered one matmul at a time between
        # attention iterations so the ACT engine's exp queue never runs dry
        # and PE never stalls (a PE stall resets the p-state ramp to half
        # clock for 3us). Tail units (normalize/transpose/out-proj) are
        # latency chains: they drain with priority, max 2 per iteration.
        work = deque()   # entries: (uid, generator)
        tails = deque()  # entries: (ready_iter, generator)
        tails2 = deque()  # out-proj units: lazy, never forced at boundaries
        est = {"pe": 0.0, "act": 0.0, "uid": 0, "done": -1, "iter": 0}
        import os
        MARGIN = float(os.environ.get("KV2_MARGIN", "0"))
        TAIL_LAG = int(os.environ.get("KV2_TAIL_LAG", "3"))
        T2_LAG = int(os.environ.get("KV2_T2_LAG", "5"))
        AHEAD = int(os.environ.get("KV2_AHEAD", "4"))
        NTAILS = int(os.environ.get("KV2_NTAILS", "1"))
        BFILL = int(os.environ.get("KV2_BFILL", "3"))

        def pump_one():
            uid, gen = work[0]
            try:
                est["pe"] += next(gen)
            except StopIteration:
                work.popleft()
                est["done"] = uid

        def pump_to_balance():
            while work and est["pe"] < est["act"] + MARGIN:
                pump_one()
            if not work and est["pe"] < est["act"]:
                # queue ran dry: the PE deficit is past idle time, not
                # fillable later — don't let it trigger a future dump
                est["pe"] = est["act"]

        def need_until(uid):
            while est["done"] < uid:
                pump_one()

        def _drain(dq, n, force):
            for _ in range(min(n, len(dq))):
                if not force and dq[0][0] > est["iter"]:
                    return
                for cost in dq.popleft()[1]:
                    est["pe"] += cost

        def drain_tails(n, force=False):
            _drain(tails, n, force)
            _drain(tails2, n, force)

        def flush_tails():
            # only tail1 (acc readers) must precede a new chunk's ctx; tail2
            # (out-proj) is exempt and drains lazily off the diag hot region
            _drain(tails, len(tails), True)

        # ---- QKV projection windows ----
        def q_unit(w, xw):
            p_ps = sm_ps.tile([P, WN], F32, name="p_ps", tag="sm")
            for kc in range(KC):
                nc.tensor.matmul(p_ps, wq_s[:, kc, :], xw[:, kc, :],
                                 start=(kc == 0), stop=(kc == KC - 1))
                yield WN * 0.42
            nc.vector.tensor_copy(qT[:, w * WN:(w + 1) * WN], p_ps)
            yield 0.0

        def k_unit(w, st, xw):
            t0 = w * WN + st * P
            p_ps = sm_ps.tile([P, P], F32, name="kp", tag="sm")
            for kc in range(KC):
                nc.tensor.matmul(p_ps, wk_s[:, kc, :],
                                 xw[:, kc, st * P:(st + 1) * P],
                                 start=(kc == 0), stop=(kc == KC - 1))
                if kc % 2 == 1:
                    yield 2 * P * 0.42
            nc.vector.tensor_copy(kT[:, t0:t0 + P], p_ps)
            yield 0.0

        def v_unit(w, st, xw):
            jt = (w * WN) // P + st
            vp = sm_ps.tile([P, CW], F32, name="vp", tag="sm")
            for kc in range(KC):
                nc.tensor.matmul(vp, xw[:, kc, st * P:(st + 1) * P],
                                 wv_s[:, kc, :],
                                 start=(kc == 0), stop=(kc == KC - 1))
                if kc % 2 == 1:
                    yield 2 * CW * 0.42
            nc.vector.tensor_copy(vA[:, jt, 0:64], vp[:, 0:64])
            nc.vector.tensor_copy(vA[:, jt, 65:129], vp[:, 64:128])
            yield 0.0

        state = {"enqueued": 0}
        uid_q = {}
        uid_k = {}
        uid_v = {}

        def add_unit(gen):
            uid = est["uid"]
            est["uid"] += 1
            work.append((uid, gen))
            return uid

        def enqueue_window(upto):
            while state["enqueued"] < min(upto, T // WN):
                w = state["enqueued"]
                xw = xw_pool.tile([P, KC, WN], BF16, name="xw", tag="xw")
                # 2 kc-chunks per dma_start: halves SP dispatch cost while
                # still spreading the window across parallel DMA engines
                for kc in range(0, KC, 2):
                    src = bass.AP(tensor=xT.tensor, offset=kc * P * T + w * WN,
                                  ap=[[T, P], [P * T, 2], [1, WN]])
                    nc.sync.dma_start(out=xw[:, kc:kc + 2, :], in_=src)
                if w == 0:
                    # deferred: not needed before the first attention chunk
                    nc.sync.dma_start(out=wo_s, in_=wo)
                    nc.sync.dma_start(out=mask_s, in_=mask)
                    nc.sync.dma_start(out=ident_s, in_=ident)
                uid_q[w] = add_unit(q_unit(w, xw))
                for st in range(WN // P):
                    gt = (w * WN) // P + st
                    uid_k[gt] = add_unit(k_unit(w, st, xw))
                    uid_v[gt] = add_unit(v_unit(w, st, xw))
                state["enqueued"] += 1

        # ---- per-query-tile tail: normalize + transpose + out-proj ----
        def tail1_unit(b, icn, qs):
            acc = accA if qs < 2 else accB
            i2 = qs % 2
            gq0 = b * S + (icn * QT + qs) * P
            rcp = rcp_sb.tile([P, 2, 1], F32, name="rcp", tag="rcp")
            nc.vector.reciprocal(rcp, acc[:, i2, :, 64:65])
            nrmd = nrm_sb.tile([P, P], BF16, name="nrmd", tag="nrmd")
            for h in range(2):
                nc.vector.tensor_scalar_mul(
                    nrmd[:, h * 64:(h + 1) * 64],
                    acc[:, i2, h, 0:64], rcp[:, h, :])
            yield 0.0
            tT = sm_ps.tile([P, P], BF16, name="tT", tag="sm")
            nc.tensor.transpose(tT, nrmd, ident_s)
            nc.vector.tensor_copy(cT[:, gq0:gq0 + P], tT)
            yield P * 0.42

        def tail2_unit(b, icn, qs, split_evac=False):
            gq0 = b * S + (icn * QT + qs) * P
            ob = ob_sb.tile([P, D], BF16, name="ob", tag="ob")
            for nn in range(D // 512):
                op = sm_ps.tile([P, 512], F32, name="op", tag="sm")
                nc.tensor.matmul(op, cT[:, gq0:gq0 + P],
                                 wo_s[:, nn * 512:(nn + 1) * 512],
                                 start=True, stop=True)
                if split_evac and nn % 2 == 0:
                    # ACT is idle at the kernel tail: split the evacuation
                    nc.scalar.activation(ob[:, nn * 512:(nn + 1) * 512], op,
                                         AF.Copy)
                else:
                    nc.vector.tensor_copy(ob[:, nn * 512:(nn + 1) * 512], op)
                yield 512 * 0.42
            # out-stores go via the idle Pool engine's SWDGE queue: their
            # data-ready waits must not block the in-order SP/HWDGE queue
            # that feeds latency-critical xw loads
            nc.gpsimd.dma_start(out=out[gq0:gq0 + P, :], in_=ob)

        # ---- attention chunks ----
        def emit_ctx(b, icn, jt, ex):
            d = jt - icn * QT
            need_until(uid_v[b * NJ + jt])
            if jt == 0:
                # new chunk touches every acc region: prior chunks' tail
                # reads must already be emitted (WAR via program order);
                # a little filler covers the DVE normalize latency so the
                # first ctx doesn't stall on the acc WAR
                flush_tails()
                for _ in range(BFILL):
                    if work:
                        pump_one()
            for h in range(2):
                for qs in range(max(d, 0), QT):
                    acc = accA if qs < 2 else accB
                    # one start=True per acc bank per chunk: start pends the
                    # whole 2KB zero region, so the other groups' first
                    # accumulate materializes zero+write (lazy per-byte).
                    # All writers are PE matmuls -> program order holds.
                    nc.tensor.matmul(
                        acc[:, qs % 2, h, :],
                        ex[:, h, qs * P:(qs + 1) * P],
                        vA[:, b * NJ + jt, h * 65:(h + 1) * 65],
                        start=(jt == 0 and h == 0 and qs % 2 == 0),
                        stop=(jt == icn * QT + qs),
                        skip_group_check=True)
                    est["pe"] += 65 * 0.42
            if d >= 0:
                last = (b == B - 1 and icn == NIC - 1)
                tails.append((est["iter"] + TAIL_LAG, tail1_unit(b, icn, d)))
                tails2.append((est["iter"] + T2_LAG,
                               tail2_unit(b, icn, d, split_evac=last)))

        # lag-2 software pipeline: ctx(jt-2) is emitted after scores(jt), so
        # PE never reaches a ctx matmul before its exp (ACT, ~1.07us after
        # its scores) has retired; pend carries across chunk boundaries
        # p-state warmup: the PE ramps to full clock only after 3us of
        # continuous execution; burn the initial xw-DMA wait on dummy
        # matmuls so the first projections run at full speed
        for i in range(9):
            wp = sm_ps.tile([P, 512], F32, name="wp", tag="sm")
            nc.tensor.matmul(wp, warm[:, 0:P], warm, start=True, stop=True)

        pend = deque()
        for b in range(B):
            for icn in range(NIC):
                # per-chunk meter reset: a cumulative surplus from the
                # PE-bound early phase must not block pre-draining filler
                # during ACT-bound chunks
                est["pe"] = est["act"] = 0.0
                gi0 = b * S + icn * IC
                njt = (icn + 1) * QT
                nw = (b * S + (icn + 1) * IC + WN - 1) // WN
                enqueue_window(nw + AHEAD)
                need_until(uid_q[(b * S + icn * IC) // WN])
                last_chunk = (b == B - 1 and icn == NIC - 1)
                for jt in range(njt):
                    est["iter"] += 1
                    # scores+exp go out first each iteration so the ACT
                    # engine's exp queue never waits behind tail/filler work
                    need_until(uid_k[b * NJ + jt])
                    il0 = max(0, jt * P - icn * IC)
                    gj0 = b * S + jt * P
                    sc = sc_ps.tile([P, 2, IC], F32, name="sc", tag="sc")
                    for h in range(2):
                        hp = h * 64
                        nc.tensor.matmul(
                            sc[:, h, il0:IC],
                            kT[hp:hp + 64, gj0:gj0 + P],
                            qT[hp:hp + 64, gi0 + il0:gi0 + IC],
                            start=True, stop=True)
                    est["pe"] += 2 * (IC - il0) * 0.42
                    ex = ex_sb.tile([P, 2, IC], BF16, name="ex", tag="ex")
                    nc.scalar.activation(ex[:, :, il0:IC], sc[:, :, il0:IC],
                                         AF.Exp, scale=0.125)
                    est["act"] += 2 * (IC - il0) * 0.84 + 185
                    if jt * P >= icn * IC:  # diagonal tile
                        for h in range(2):
                            nc.vector.tensor_mul(ex[:, h, il0:il0 + P],
                                                 ex[:, h, il0:il0 + P],
                                                 mask_s)
                    if last_chunk:
                        # no later chunk will absorb the backlog: drain all
                        # ready tails now so the end tail stays short
                        drain_tails(4)
                        if jt >= njt - 8:
                            flush_tails()
                            _drain(tails2, len(tails2), True)
                    else:
                        drain_tails(NTAILS)
                    pend.append((b, icn, jt, ex))
                    if len(pend) > 2:
                        emit_ctx(*pend.popleft())
                    pump_to_balance()
                # drain the ctx pipeline at chunk end so the last tiles'
                # tail1 normalizes get a couple of iterations of slack
                # before the next chunk's first ctx (acc WAR); filler first
                # so the last ctx doesn't stall on its just-emitted exp
                for _ in range(BFILL):
                    if work:
                        pump_one()
                while pend:
                    emit_ctx(*pend.popleft())
        flush_tails()
        _drain(tails2, len(tails2), True)
        while work:
            pump_one()

    nc.compile()
    return nc


def _warrange(w, bf16):
    # [D, CW] -> [P, D//P, CW] contiguous (the SBUF layout, so the DMA is
    # a single contiguous copy instead of 256B strided pieces)
    D, CW_ = w.shape
    return np.ascontiguousarray(
        w.reshape(D // P, P, CW_).transpose(1, 0, 2)).astype(bf16)


def make_in_maps(x, Wq, Wk, Wv, Wo):
    import ml_dtypes
    bf16 = ml_dtypes.bfloat16
    B, S, D = x.shape
    xT = np.ascontiguousarray(x.reshape(B * S, D).T).astype(bf16)
    mask = np.triu(np.ones((P, P), dtype=bf16))
    ident = np.eye(P, dtype=bf16)
    in_maps = []
    for c in range(N_CORES):
        cs = slice(c * CW, (c + 1) * CW)
        in_maps.append({
            "xT": xT,
            "wq": _warrange(Wq[:, cs], bf16),
            "wk": _warrange(Wk[:, cs], bf16),
            "wv": _warrange(Wv[:, cs], bf16),
            "wo": np.ascontiguousarray(Wo[cs, :]).astype(bf16),
            "mask": mask,
            "ident": ident,
        })
    return in_maps


_CACHED_NC = None


def kernel(x, Wq, Wk, Wv, Wo, bo, _trace=False):
    global _CACHED_NC
    x = np.asarray(x, dtype=np.float32)
    B, S, D = x.shape
    if _CACHED_NC is None:
        _CACHED_NC = build_program(S=S, B=B, D=D)
    nc = _CACHED_NC
    in_maps = make_in_maps(x, np.asarray(Wq), np.asarray(Wk),
                           np.asarray(Wv), np.asarray(Wo))
    res = None
    for attempt in range(3):
        try:
            res = run_bass_kernel_spmd(nc, in_maps, list(range(N_CORES)),
                                       trace=_trace)
            break
        except Exception:
            if attempt == 2:
                raise
    out = np.zeros((B * S, D), dtype=np.float32)
    for c in range(N_CORES):
        out += res.results[c]["out"].astype(np.float32)
    out += np.asarray(bo, dtype=np.float32)[None, :]
    if _trace:
        kernel._last_result = res
    return out.reshape(B, S, D)


# revision 3
# speedup vs baseline: 1.2787x; 1.0887x over previous
"""Multi-head causal attention (B=2, S=4096, D=1024, H=16) on 8 TRN2 NeuronCores.

Sharding: head-parallel. Core c computes heads 2c, 2c+1 (128 of the 1024
projection columns) for both batches:
  - QKV column-parallel: each core gets Wq/Wk/Wv[:, c*128:(c+1)*128]
  - out-proj row-parallel: partial_out = ctx_c @ Wo[c*128:(c+1)*128, :]
  - host sums the 8 partials and adds bo.

The TimelineSim cost model charges a matmul only by its OUTPUT free size
(N columns), so the ctx product is blocked transposed: per (128-query x
128-key) causal block, out[128 q, 65] += ex[128 k, 128 q].T @ vA[128 k, 65]
streams just 65 columns (the 65th is an all-ones column producing the
softmax denominator), instead of re-streaming 512 query columns per key
tile. The denominator then sits on the free dim, so normalization is a
cheap per-partition tensor_scalar, and a single PE transpose per query
tile restores the [cdim, token] layout for the out-projection.

Layouts on-chip (per core):
  qT, kT:  [128, T]  rows 0:64 head0, 64:128 head1 (transposed projections)
  vA:      [128, T/128, 130]  per key-tile [v_h0 | ones | v_h1 | ones]
  sc/ex:   [128 keys, 2 heads, 512 queries]  PSUM scores / SBUF exp(bf16)
  acc:     PSUM [128 q, 2 qsub, 2 heads, 65] x2 (unnormalized ctx + denom)
  cT:      [128 cdim, T]  normalized ctx, transposed back per 128-q tile

Emission keeps both bottleneck engines (PE ~283us of matmul columns,
ACT ~280us of exp) near-continuously busy: QKV projection windows are
generators on a work queue, metered one matmul at a time between
attention iterations (forced just-in-time by fine-grained need-barriers,
pre-drained during ACT-bound chunks); per-tile tail work drains with an
iteration lag so its cross-engine latency chains stay off the PE stream;
out-stores ride the idle Pool engine's SWDGE queue so their data waits
never block the in-order SP queue feeding xw loads; deep SBUF pools
(ex/nrm/rcp/ob) keep WAR rotations off the critical path; dummy warm-up
matmuls ramp the PE p-state during the initial DMA fill.
"""

from collections import deque
from contextlib import ExitStack

import numpy as np

import concourse.bass as bass
import concourse.tile as tile
from concourse import bacc, mybir
from concourse.bass_utils import run_bass_kernel_spmd

F32 = mybir.dt.float32
BF16 = mybir.dt.bfloat16
P = 128
AF = mybir.ActivationFunctionType

N_CORES = 8
B_FULL, S_FULL, D_FULL, H_FULL = 2, 4096, 1024, 16
DH = 64
CW = 128  # projection columns per core (2 heads * 64)


def build_program(S=S_FULL, B=B_FULL, D=D_FULL):
    """Build the per-core Bass program (same program on all 8 cores)."""
    T = B * S
    KC = D // P            # contraction chunks for the projections
    IC = min(512, S)       # query-chunk width
    QT = IC // P           # query tiles per chunk (4)
    NJ = S // P            # key tiles per batch
    NIC = S // IC          # query chunks per batch
    WN = min(512, T)       # QKV token window

    nc = bacc.Bacc("TRN2", target_bir_lowering=False, debug=False,
                   num_devices=N_CORES)

    xT = nc.dram_tensor("xT", [D, T], BF16, kind="ExternalInput").ap()
    wq = nc.dram_tensor("wq", [P, D // P, CW], BF16, kind="ExternalInput").ap()
    wk = nc.dram_tensor("wk", [P, D // P, CW], BF16, kind="ExternalInput").ap()
    wv = nc.dram_tensor("wv", [P, D // P, CW], BF16, kind="ExternalInput").ap()
    wo = nc.dram_tensor("wo", [CW, D], BF16, kind="ExternalInput").ap()
    mask = nc.dram_tensor("mask", [P, P], BF16, kind="ExternalInput").ap()
    ident = nc.dram_tensor("ident", [P, P], BF16, kind="ExternalInput").ap()
    out = nc.dram_tensor("out", [T, D], BF16, kind="ExternalOutput").ap()

    with tile.TileContext(nc) as tc, ExitStack() as ctx:
        singles = ctx.enter_context(tc.tile_pool(name="singles", bufs=1))
        qT = singles.tile([P, T], BF16, name="qT")
        kT = singles.tile([P, T], BF16, name="kT")
        vA = singles.tile([P, B * NJ, 130], BF16, name="vA")
        cT = singles.tile([P, T], BF16, name="cT")
        wq_s = singles.tile([P, KC, CW], BF16, name="wq_s")
        wk_s = singles.tile([P, KC, CW], BF16, name="wk_s")
        wv_s = singles.tile([P, KC, CW], BF16, name="wv_s")
        wo_s = singles.tile([CW, D], BF16, name="wo_s")
        mask_s = singles.tile([P, P], BF16, name="mask_s")
        ident_s = singles.tile([P, P], BF16, name="ident_s")

        # weight loads dispatch on the ACT hwdge queue, in parallel with
        # the SP queue's first xw window loads
        nc.scalar.dma_start(out=wq_s, in_=wq)
        nc.scalar.dma_start(out=wk_s, in_=wk)
        nc.scalar.dma_start(out=wv_s, in_=wv)
        nc.vector.memset(vA[:, :, 64:65], 1.0)
        nc.vector.memset(vA[:, :, 129:130], 1.0)
        warm = singles.tile([P, 512], BF16, name="warm")
        nc.vector.memset(warm, 0.0)

        # PSUM budget (8 banks): sc 2x2 + acc 2x1 + shared-small 2x1 = 8
        ps_single = ctx.enter_context(
            tc.tile_pool(name="ps_single", bufs=1, space=bass.MemorySpace.PSUM))
        # [q, qsub-pair, head, v+denom]; accA holds qsub 0,1; accB 2,3
        accA = ps_single.tile([P, 2, 2, 65], F32, name="accA")
        accB = ps_single.tile([P, 2, 2, 65], F32, name="accB")
        sc_ps = ctx.enter_context(
            tc.tile_pool(name="sc_ps", bufs=2, space=bass.MemorySpace.PSUM))
        sm_ps = ctx.enter_context(
            tc.tile_pool(name="sm_ps", bufs=2, space=bass.MemorySpace.PSUM))
        xw_pool = ctx.enter_context(tc.tile_pool(name="xw_pool", bufs=10))
        ex_sb = ctx.enter_context(tc.tile_pool(name="ex_sb", bufs=8))
        nrm_sb = ctx.enter_context(tc.tile_pool(name="nrm_sb", bufs=8))
        rcp_sb = ctx.enter_context(tc.tile_pool(name="rcp_sb", bufs=8))
        ob_sb = ctx.enter_context(tc.tile_pool(name="ob_sb", bufs=4))

        # ---- deferred-work queue of generators yielding ~PE-ns steps ----
        # Filler (QKV windows) is metered one matmul at a time between
        # attention iterations so the ACT engine's exp queue never runs dry
        # and PE never stalls (a PE stall resets the p-state ramp to half
        # clock for 3us). Tail units (normalize/transpose/out-proj) are
        # latency chains: they drain with priority, max 2 per iteration.
        import os
        work = deque()   # entries: (uid, generator)
        tails = deque()  # entries: (ready_iter, generator)
        tails2 = deque()  # out-proj units: lazy, never forced at boundaries
        est = {"pe": 0.0, "act": 0.0, "uid": 0, "done": -1, "iter": 0}
        MARGIN = float(os.environ.get("KV2_MARGIN", "0"))
        TAIL_LAG = int(os.environ.get("KV2_TAIL_LAG", "4"))
        T2_LAG = int(os.environ.get("KV2_T2_LAG", "5"))
        AHEAD = int(os.environ.get("KV2_AHEAD", "4"))
        NTAILS = int(os.environ.get("KV2_NTAILS", "2"))
        BFILL = int(os.environ.get("KV2_BFILL", "3"))

        def pump_one():
            uid, gen = work[0]
            try:
                est["pe"] += next(gen)
            except StopIteration:
                work.popleft()
                est["done"] = uid

        def pump_to_balance():
            while work and est["pe"] < est["act"] + MARGIN:
                pump_one()
            if not work and est["pe"] < est["act"]:
                # queue ran dry: the PE deficit is past idle time, not
                # fillable later — don't let it trigger a future dump
                est["pe"] = est["act"]

        def need_until(uid):
            while est["done"] < uid:
                pump_one()

        def _drain(dq, n, force):
            for _ in range(min(n, len(dq))):
                if not force and dq[0][0] > est["iter"]:
                    return
                for cost in dq.popleft()[1]:
                    est["pe"] += cost

        def drain_tails(n, force=False):
            _drain(tails, n, force)
            _drain(tails2, n, force)

        def flush_tails():
            # only tail1 (acc readers) must precede a new chunk's ctx; tail2
            # (out-proj) is exempt and drains lazily off the diag hot region
            _drain(tails, len(tails), True)

        # ---- QKV projection windows ----
        def q_unit(w, xw):
            p_ps = sm_ps.tile([P, WN], F32, name="p_ps", tag="sm")
            for kc in range(KC):
                nc.tensor.matmul(p_ps, wq_s[:, kc, :], xw[:, kc, :],
                                 start=(kc == 0), stop=(kc == KC - 1))
                yield WN * 0.42
            nc.vector.tensor_copy(qT[:, w * WN:(w + 1) * WN], p_ps)
            yield 0.0

        def k_unit(w, st, xw):
            t0 = w * WN + st * P
            p_ps = sm_ps.tile([P, P], F32, name="kp", tag="sm")
            for kc in range(KC):
                nc.tensor.matmul(p_ps, wk_s[:, kc, :],
                                 xw[:, kc, st * P:(st + 1) * P],
                                 start=(kc == 0), stop=(kc == KC - 1))
                if kc % 2 == 1:
                    yield 2 * P * 0.42
            nc.vector.tensor_copy(kT[:, t0:t0 + P], p_ps)
            yield 0.0

        def v_unit(w, st, xw):
            jt = (w * WN) // P + st
            vp = sm_ps.tile([P, CW], F32, name="vp", tag="sm")
            for kc in range(KC):
                nc.tensor.matmul(vp, xw[:, kc, st * P:(st + 1) * P],
                                 wv_s[:, kc, :],
                                 start=(kc == 0), stop=(kc == KC - 1))
                if kc % 2 == 1:
                    yield 2 * CW * 0.42
            nc.vector.tensor_copy(vA[:, jt, 0:64], vp[:, 0:64])
            nc.vector.tensor_copy(vA[:, jt, 65:129], vp[:, 64:128])
            yield 0.0

        state = {"enqueued": 0}
        uid_q = {}
        uid_k = {}
        uid_v = {}

        def add_unit(gen):
            uid = est["uid"]
            est["uid"] += 1
            work.append((uid, gen))
            return uid

        def enqueue_window(upto):
            while state["enqueued"] < min(upto, T // WN):
                w = state["enqueued"]
                xw = xw_pool.tile([P, KC, WN], BF16, name="xw", tag="xw")
                # 2 kc-chunks per dma_start: halves SP dispatch cost while
                # still spreading the window across parallel DMA engines
                for kc in range(0, KC, 2):
                    src = bass.AP(tensor=xT.tensor, offset=kc * P * T + w * WN,
                                  ap=[[T, P], [P * T, 2], [1, WN]])
                    nc.sync.dma_start(out=xw[:, kc:kc + 2, :], in_=src)
                if w == 0:
                    # deferred: not needed before the first attention chunk
                    nc.scalar.dma_start(out=wo_s, in_=wo)
                    nc.scalar.dma_start(out=mask_s, in_=mask)
                    nc.scalar.dma_start(out=ident_s, in_=ident)
                uid_q[w] = add_unit(q_unit(w, xw))
                for st in range(WN // P):
                    gt = (w * WN) // P + st
                    uid_k[gt] = add_unit(k_unit(w, st, xw))
                    uid_v[gt] = add_unit(v_unit(w, st, xw))
                state["enqueued"] += 1

        # ---- per-query-tile tail: normalize + transpose + out-proj ----
        def tail1_unit(b, icn, qs):
            acc = accA if qs < 2 else accB
            i2 = qs % 2
            gq0 = b * S + (icn * QT + qs) * P
            rcp = rcp_sb.tile([P, 2, 1], F32, name="rcp", tag="rcp")
            nc.vector.reciprocal(rcp, acc[:, i2, :, 64:65])
            nrmd = nrm_sb.tile([P, P], BF16, name="nrmd", tag="nrmd")
            for h in range(2):
                nc.vector.tensor_scalar_mul(
                    nrmd[:, h * 64:(h + 1) * 64],
                    acc[:, i2, h, 0:64], rcp[:, h, :])
            yield 0.0
            tT = sm_ps.tile([P, P], BF16, name="tT", tag="sm")
            nc.tensor.transpose(tT, nrmd, ident_s)
            nc.vector.tensor_copy(cT[:, gq0:gq0 + P], tT)
            yield P * 0.42

        def tail2_unit(b, icn, qs, split_evac=False):
            gq0 = b * S + (icn * QT + qs) * P
            ob = ob_sb.tile([P, D], BF16, name="ob", tag="ob")
            for nn in range(D // 512):
                op = sm_ps.tile([P, 512], F32, name="op", tag="sm")
                nc.tensor.matmul(op, cT[:, gq0:gq0 + P],
                                 wo_s[:, nn * 512:(nn + 1) * 512],
                                 start=True, stop=True)
                if split_evac and nn % 2 == 0:
                    # ACT is idle at the kernel tail: split the evacuation
                    nc.scalar.activation(ob[:, nn * 512:(nn + 1) * 512], op,
                                         AF.Copy)
                else:
                    nc.vector.tensor_copy(ob[:, nn * 512:(nn + 1) * 512], op)
                yield 512 * 0.42
            # out-stores go via the idle Pool engine's SWDGE queue: their
            # data-ready waits must not block the in-order SP/HWDGE queue
            # that feeds latency-critical xw loads
            nc.gpsimd.dma_start(out=out[gq0:gq0 + P, :], in_=ob)

        # ---- attention chunks ----
        def emit_ctx(b, icn, jt, ex):
            d = jt - icn * QT
            need_until(uid_v[b * NJ + jt])
            if jt == 0:
                # new chunk touches every acc region: prior chunks' tail
                # reads must already be emitted (WAR via program order);
                # a little filler covers the DVE normalize latency so the
                # first ctx doesn't stall on the acc WAR
                flush_tails()
                for _ in range(BFILL):
                    if work:
                        pump_one()
            for h in range(2):
                for qs in range(max(d, 0), QT):
                    acc = accA if qs < 2 else accB
                    # one start=True per acc bank per chunk: start pends the
                    # whole 2KB zero region, so the other groups' first
                    # accumulate materializes zero+write (lazy per-byte).
                    # All writers are PE matmuls -> program order holds.
                    nc.tensor.matmul(
                        acc[:, qs % 2, h, :],
                        ex[:, h, qs * P:(qs + 1) * P],
                        vA[:, b * NJ + jt, h * 65:(h + 1) * 65],
                        start=(jt == 0 and h == 0 and qs % 2 == 0),
                        stop=(jt == icn * QT + qs),
                        skip_group_check=True)
                    est["pe"] += 65 * 0.42
            if d >= 0:
                last = (b == B - 1 and icn == NIC - 1)
                tails.append((est["iter"] + TAIL_LAG, tail1_unit(b, icn, d)))
                tails2.append((est["iter"] + T2_LAG,
                               tail2_unit(b, icn, d, split_evac=last)))

        # lag-2 software pipeline: ctx(jt-2) is emitted after scores(jt), so
        # PE never reaches a ctx matmul before its exp (ACT, ~1.07us after
        # its scores) has retired; pend carries across chunk boundaries
        # p-state warmup: the PE ramps to full clock only after 3us of
        # continuous execution; burn the initial xw-DMA wait on dummy
        # matmuls so the first projections run at full speed
        for i in range(4):
            wp = sm_ps.tile([P, 512], F32, name="wp", tag="sm")
            nc.tensor.matmul(wp, warm[:, 0:P], warm, start=True, stop=True)

        pend = deque()
        for b in range(B):
            for icn in range(NIC):
                # per-chunk meter reset: a cumulative surplus from the
                # PE-bound early phase must not block pre-draining filler
                # during ACT-bound chunks
                est["pe"] = est["act"] = 0.0
                gi0 = b * S + icn * IC
                njt = (icn + 1) * QT
                nw = (b * S + (icn + 1) * IC + WN - 1) // WN
                enqueue_window(nw + AHEAD)
                need_until(uid_q[(b * S + icn * IC) // WN])
                last_chunk = (b == B - 1 and icn == NIC - 1)
                for jt in range(njt):
                    est["iter"] += 1
                    if icn >= 5:
                        # ACT-bound chunk: filler first, so PE doesn't sit
                        # stalled on the sc-buffer WAR (exp jt-2) and reset
                        # its p-state ramp
                        pump_to_balance()
                    # scores+exp go out first each iteration so the ACT
                    # engine's exp queue never waits behind tail/filler work
                    need_until(uid_k[b * NJ + jt])
                    il0 = max(0, jt * P - icn * IC)
                    gj0 = b * S + jt * P
                    sc = sc_ps.tile([P, 2, IC], F32, name="sc", tag="sc")
                    for h in range(2):
                        hp = h * 64
                        nc.tensor.matmul(
                            sc[:, h, il0:IC],
                            kT[hp:hp + 64, gj0:gj0 + P],
                            qT[hp:hp + 64, gi0 + il0:gi0 + IC],
                            start=True, stop=True)
                    est["pe"] += 2 * (IC - il0) * 0.42
                    ex = ex_sb.tile([P, 2, IC], BF16, name="ex", tag="ex")
                    nc.scalar.activation(ex[:, :, il0:IC], sc[:, :, il0:IC],
                                         AF.Exp, scale=0.125)
                    est["act"] += 2 * (IC - il0) * 0.84 + 185
                    if jt * P >= icn * IC:  # diagonal tile
                        for h in range(2):
                            nc.vector.tensor_mul(ex[:, h, il0:il0 + P],
                                                 ex[:, h, il0:il0 + P],
                                                 mask_s)
                    if last_chunk:
                        # no later chunk will absorb the backlog: drain all
                        # ready tails now so the end tail stays short
                        drain_tails(4)
                        if jt >= njt - 8:
                            flush_tails()
                            _drain(tails2, len(tails2), True)
                    else:
                        drain_tails(NTAILS)
                    pend.append((b, icn, jt, ex))
                    # deeper ctx lag in short early chunks: gives the prev
                    # chunk's last tail1 normalize more slack before the
                    # first ctx of this chunk hits the acc WAR
                    plag = 3 if icn <= int(os.environ.get('KV2_PLAGC', '7')) else 2
                    if len(pend) > plag:
                        emit_ctx(*pend.popleft())
                    pump_to_balance()
                # drain the ctx pipeline at chunk end so the last tiles'
                # tail1 normalizes get a couple of iterations of slack
                # before the next chunk's first ctx (acc WAR); filler first
                # so the last ctx doesn't stall on its just-emitted exp
                for _ in range(BFILL):
                    if work:
                        pump_one()
                while pend:
                    emit_ctx(*pend.popleft())
        flush_tails()
        _drain(tails2, len(tails2), True)
        while work:
            pump_one()

    nc.compile()
    return nc


def _warrange(w, bf16):
    # [D, CW] -> [P, D//P, CW] contiguous (the SBUF layout, so the DMA is
    # a single contiguous copy instead of 256B strided pieces)
    D, CW_ = w.shape
    return np.ascontiguousarray(
        w.reshape(D // P, P, CW_).transpose(1, 0, 2)).astype(bf16)


def make_in_maps(x, Wq, Wk, Wv, Wo):
    import ml_dtypes
    bf16 = ml_dtypes.bfloat16
    B, S, D = x.shape
    xT = np.ascontiguousarray(x.reshape(B * S, D).T).astype(bf16)
    mask = np.triu(np.ones((P, P), dtype=bf16))
    ident = np.eye(P, dtype=bf16)
    in_maps = []
    for c in range(N_CORES):
        cs = slice(c * CW, (c + 1) * CW)
        in_maps.append({
            "xT": xT,
            "wq": _warrange(Wq[:, cs], bf16),
            "wk": _warrange(Wk[:, cs], bf16),
            "wv": _warrange(Wv[:, cs], bf16),
            "wo": np.ascontiguousarray(Wo[cs, :]).astype(bf16),
            "mask": mask,
            "ident": ident,
        })
    return in_maps


_CACHED_NC = None


def kernel(x, Wq, Wk, Wv, Wo, bo, _trace=False):
    global _CACHED_NC
    x = np.asarray(x, dtype=np.float32)
    B, S, D = x.shape
    if _CACHED_NC is None:
        _CACHED_NC = build_program(S=S, B=B, D=D)
    nc = _CACHED_NC
    in_maps = make_in_maps(x, np.asarray(Wq), np.asarray(Wk),
                           np.asarray(Wv), np.asarray(Wo))
    res = None
    for attempt in range(3):
        try:
            res = run_bass_kernel_spmd(nc, in_maps, list(range(N_CORES)),
                                       trace=_trace)
            break
        except Exception:
            if attempt == 2:
                raise
    out = np.zeros((B * S, D), dtype=np.float32)
    for c in range(N_CORES):
        out += res.results[c]["out"].astype(np.float32)
    out += np.asarray(bo, dtype=np.float32)[None, :]
    if _trace:
        kernel._last_result = res
    return out.reshape(B, S, D)


# revision 4
# speedup vs baseline: 1.2875x; 1.0068x over previous
"""Multi-head causal attention (B=2, S=4096, D=1024, H=16) on 8 TRN2 NeuronCores.

Sharding: head-parallel. Core c computes heads 2c, 2c+1 (128 of the 1024
projection columns) for both batches:
  - QKV column-parallel: each core gets Wq/Wk/Wv[:, c*128:(c+1)*128]
  - out-proj row-parallel: partial_out = ctx_c @ Wo[c*128:(c+1)*128, :]
  - host sums the 8 partials and adds bo.

The TimelineSim cost model charges a matmul only by its OUTPUT free size
(N columns), so the ctx product is blocked transposed: per (128-query x
128-key) causal block, out[128 q, 65] += ex[128 k, 128 q].T @ vA[128 k, 65]
streams just 65 columns (the 65th is an all-ones column producing the
softmax denominator), instead of re-streaming 512 query columns per key
tile. The denominator then sits on the free dim, so normalization is a
cheap per-partition tensor_scalar, and a single PE transpose per query
tile restores the [cdim, token] layout for the out-projection.

Layouts on-chip (per core):
  qT, kT:  [128, T]  rows 0:64 head0, 64:128 head1 (transposed projections)
  vA:      [128, T/128, 130]  per key-tile [v_h0 | ones | v_h1 | ones]
  sc/ex:   [128 keys, 2 heads, 512 queries]  PSUM scores / SBUF exp(bf16)
  acc:     PSUM [128 q, 2 qsub, 2 heads, 65] x2 (unnormalized ctx + denom)
  cT:      [128 cdim, T]  normalized ctx, transposed back per 128-q tile

Emission keeps both bottleneck engines (PE ~283us of matmul columns,
ACT ~280us of exp) near-continuously busy: QKV projection windows are
generators on a work queue, metered one matmul at a time between
attention iterations (forced just-in-time by fine-grained need-barriers,
pre-drained during ACT-bound chunks); per-tile tail work drains with an
iteration lag so its cross-engine latency chains stay off the PE stream;
out-stores ride the idle Pool engine's SWDGE queue so their data waits
never block the in-order SP queue feeding xw loads; deep SBUF pools
(ex/nrm/rcp/ob) keep WAR rotations off the critical path; dummy warm-up
matmuls ramp the PE p-state during the initial DMA fill.
"""

from collections import deque
from contextlib import ExitStack

import numpy as np

import concourse.bass as bass
import concourse.tile as tile
from concourse import bacc, mybir
from concourse.bass_utils import run_bass_kernel_spmd

F32 = mybir.dt.float32
BF16 = mybir.dt.bfloat16
P = 128
AF = mybir.ActivationFunctionType

N_CORES = 8
B_FULL, S_FULL, D_FULL, H_FULL = 2, 4096, 1024, 16
DH = 64
CW = 128  # projection columns per core (2 heads * 64)


def build_program(S=S_FULL, B=B_FULL, D=D_FULL):
    """Build the per-core Bass program (same program on all 8 cores)."""
    T = B * S
    KC = D // P            # contraction chunks for the projections
    IC = min(512, S)       # query-chunk width
    QT = IC // P           # query tiles per chunk (4)
    NJ = S // P            # key tiles per batch
    NIC = S // IC          # query chunks per batch
    WN = min(512, T)       # QKV token window

    nc = bacc.Bacc("TRN2", target_bir_lowering=False, debug=False,
                   num_devices=N_CORES)

    xT = nc.dram_tensor("xT", [D, T], BF16, kind="ExternalInput").ap()
    wq = nc.dram_tensor("wq", [P, D // P, CW], BF16, kind="ExternalInput").ap()
    wk = nc.dram_tensor("wk", [P, D // P, CW], BF16, kind="ExternalInput").ap()
    wv = nc.dram_tensor("wv", [P, D // P, CW], BF16, kind="ExternalInput").ap()
    wo = nc.dram_tensor("wo", [CW, D], BF16, kind="ExternalInput").ap()
    mask = nc.dram_tensor("mask", [P, P], BF16, kind="ExternalInput").ap()
    ident = nc.dram_tensor("ident", [P, P], BF16, kind="ExternalInput").ap()
    out = nc.dram_tensor("out", [T, D], BF16, kind="ExternalOutput").ap()

    with tile.TileContext(nc) as tc, ExitStack() as ctx:
        singles = ctx.enter_context(tc.tile_pool(name="singles", bufs=1))
        qT = singles.tile([P, T], BF16, name="qT")
        kT = singles.tile([P, T], BF16, name="kT")
        vA = singles.tile([P, B * NJ, 130], BF16, name="vA")
        cT = singles.tile([P, T], BF16, name="cT")
        wq_s = singles.tile([P, KC, CW], BF16, name="wq_s")
        wk_s = singles.tile([P, KC, CW], BF16, name="wk_s")
        wv_s = singles.tile([P, KC, CW], BF16, name="wv_s")
        wo_s = singles.tile([CW, D], BF16, name="wo_s")
        mask_s = singles.tile([P, P], BF16, name="mask_s")
        ident_s = singles.tile([P, P], BF16, name="ident_s")

        # weight loads dispatch on the ACT hwdge queue, in parallel with
        # the SP queue's first xw window loads
        nc.scalar.dma_start(out=wq_s, in_=wq)
        nc.scalar.dma_start(out=wk_s, in_=wk)
        nc.scalar.dma_start(out=wv_s, in_=wv)
        nc.vector.memset(vA[:, :, 64:65], 1.0)
        nc.vector.memset(vA[:, :, 129:130], 1.0)
        warm = singles.tile([P, 512], BF16, name="warm")
        nc.vector.memset(warm, 0.0)

        # PSUM budget (8 banks): sc 2x2 + acc 2x1 + shared-small 2x1 = 8
        ps_single = ctx.enter_context(
            tc.tile_pool(name="ps_single", bufs=1, space=bass.MemorySpace.PSUM))
        # [q, qsub-pair, head, v+denom]; accA holds qsub 0,1; accB 2,3
        accA = ps_single.tile([P, 2, 2, 65], F32, name="accA")
        accB = ps_single.tile([P, 2, 2, 65], F32, name="accB")
        sc_ps = ctx.enter_context(
            tc.tile_pool(name="sc_ps", bufs=2, space=bass.MemorySpace.PSUM))
        sm_ps = ctx.enter_context(
            tc.tile_pool(name="sm_ps", bufs=2, space=bass.MemorySpace.PSUM))
        xw_pool = ctx.enter_context(tc.tile_pool(name="xw_pool", bufs=10))
        ex_sb = ctx.enter_context(tc.tile_pool(name="ex_sb", bufs=8))
        nrm_sb = ctx.enter_context(tc.tile_pool(name="nrm_sb", bufs=8))
        rcp_sb = ctx.enter_context(tc.tile_pool(name="rcp_sb", bufs=8))
        ob_sb = ctx.enter_context(tc.tile_pool(name="ob_sb", bufs=4))

        # ---- deferred-work queue of generators yielding ~PE-ns steps ----
        # Filler (QKV windows) is metered one matmul at a time between
        # attention iterations so the ACT engine's exp queue never runs dry
        # and PE never stalls (a PE stall resets the p-state ramp to half
        # clock for 3us). Tail units (normalize/transpose/out-proj) are
        # latency chains: they drain with priority, max 2 per iteration.
        import os
        work = deque()   # entries: (uid, generator)
        tails = deque()  # entries: (ready_iter, generator)
        tails2 = deque()  # out-proj units: lazy, never forced at boundaries
        est = {"pe": 0.0, "act": 0.0, "uid": 0, "done": -1, "iter": 0}
        MARGIN = float(os.environ.get("KV2_MARGIN", "0"))
        TAIL_LAG = int(os.environ.get("KV2_TAIL_LAG", "4"))
        T2_LAG = int(os.environ.get("KV2_T2_LAG", "5"))
        AHEAD = int(os.environ.get("KV2_AHEAD", "4"))
        NTAILS = int(os.environ.get("KV2_NTAILS", "2"))
        BFILL = int(os.environ.get("KV2_BFILL", "0"))

        def pump_one():
            uid, gen = work[0]
            try:
                est["pe"] += next(gen)
            except StopIteration:
                work.popleft()
                est["done"] = uid

        def pump_to_balance():
            while work and est["pe"] < est["act"] + MARGIN:
                pump_one()
            if not work and est["pe"] < est["act"]:
                # queue ran dry: the PE deficit is past idle time, not
                # fillable later — don't let it trigger a future dump
                est["pe"] = est["act"]

        def need_until(uid):
            while est["done"] < uid:
                pump_one()

        def _drain(dq, n, force):
            for _ in range(min(n, len(dq))):
                if not force and dq[0][0] > est["iter"]:
                    return
                for cost in dq.popleft()[1]:
                    est["pe"] += cost

        def drain_tails(n, force=False):
            _drain(tails, n, force)
            _drain(tails2, n, force)

        def flush_tails():
            # only tail1 (acc readers) must precede a new chunk's ctx; tail2
            # (out-proj) is exempt and drains lazily off the diag hot region
            _drain(tails, len(tails), True)

        # ---- QKV projection windows ----
        def q_unit(w, xw):
            p_ps = sm_ps.tile([P, WN], F32, name="p_ps", tag="sm")
            for kc in range(KC):
                nc.tensor.matmul(p_ps, wq_s[:, kc, :], xw[:, kc, :],
                                 start=(kc == 0), stop=(kc == KC - 1))
                yield WN * 0.42
            nc.vector.tensor_copy(qT[:, w * WN:(w + 1) * WN], p_ps)
            yield 0.0

        def k_unit(w, st, xw):
            t0 = w * WN + st * P
            p_ps = sm_ps.tile([P, P], F32, name="kp", tag="sm")
            for kc in range(KC):
                nc.tensor.matmul(p_ps, wk_s[:, kc, :],
                                 xw[:, kc, st * P:(st + 1) * P],
                                 start=(kc == 0), stop=(kc == KC - 1))
                if kc % 2 == 1:
                    yield 2 * P * 0.42
            nc.vector.tensor_copy(kT[:, t0:t0 + P], p_ps)
            yield 0.0

        def v_unit(w, st, xw):
            jt = (w * WN) // P + st
            vp = sm_ps.tile([P, CW], F32, name="vp", tag="sm")
            for kc in range(KC):
                nc.tensor.matmul(vp, xw[:, kc, st * P:(st + 1) * P],
                                 wv_s[:, kc, :],
                                 start=(kc == 0), stop=(kc == KC - 1))
                if kc % 2 == 1:
                    yield 2 * CW * 0.42
            nc.vector.tensor_copy(vA[:, jt, 0:64], vp[:, 0:64])
            nc.vector.tensor_copy(vA[:, jt, 65:129], vp[:, 64:128])
            yield 0.0

        state = {"enqueued": 0}
        uid_q = {}
        uid_k = {}
        uid_v = {}

        def add_unit(gen):
            uid = est["uid"]
            est["uid"] += 1
            work.append((uid, gen))
            return uid

        def enqueue_window(upto):
            while state["enqueued"] < min(upto, T // WN):
                w = state["enqueued"]
                xw = xw_pool.tile([P, KC, WN], BF16, name="xw", tag="xw")
                # 2 kc-chunks per dma_start: halves SP dispatch cost while
                # still spreading the window across parallel DMA engines
                for kc in range(0, KC, 2):
                    src = bass.AP(tensor=xT.tensor, offset=kc * P * T + w * WN,
                                  ap=[[T, P], [P * T, 2], [1, WN]])
                    nc.sync.dma_start(out=xw[:, kc:kc + 2, :], in_=src)
                if w == 0:
                    # deferred: not needed before the first attention chunk
                    nc.scalar.dma_start(out=wo_s, in_=wo)
                    nc.scalar.dma_start(out=mask_s, in_=mask)
                    nc.scalar.dma_start(out=ident_s, in_=ident)
                uid_q[w] = add_unit(q_unit(w, xw))
                for st in range(WN // P):
                    gt = (w * WN) // P + st
                    uid_k[gt] = add_unit(k_unit(w, st, xw))
                    uid_v[gt] = add_unit(v_unit(w, st, xw))
                state["enqueued"] += 1

        # ---- per-query-tile tail: normalize + transpose + out-proj ----
        def tail1_unit(b, icn, qs):
            acc = accA if qs < 2 else accB
            i2 = qs % 2
            gq0 = b * S + (icn * QT + qs) * P
            rcp = rcp_sb.tile([P, 2, 1], F32, name="rcp", tag="rcp")
            nc.vector.reciprocal(rcp, acc[:, i2, :, 64:65])
            nrmd = nrm_sb.tile([P, P], BF16, name="nrmd", tag="nrmd")
            for h in range(2):
                nc.vector.tensor_scalar_mul(
                    nrmd[:, h * 64:(h + 1) * 64],
                    acc[:, i2, h, 0:64], rcp[:, h, :])
            yield 0.0
            tT = sm_ps.tile([P, P], BF16, name="tT", tag="sm")
            nc.tensor.transpose(tT, nrmd, ident_s)
            nc.vector.tensor_copy(cT[:, gq0:gq0 + P], tT)
            yield P * 0.42

        def tail2_unit(b, icn, qs, split_evac=False):
            gq0 = b * S + (icn * QT + qs) * P
            ob = ob_sb.tile([P, D], BF16, name="ob", tag="ob")
            for nn in range(D // 512):
                op = sm_ps.tile([P, 512], F32, name="op", tag="sm")
                nc.tensor.matmul(op, cT[:, gq0:gq0 + P],
                                 wo_s[:, nn * 512:(nn + 1) * 512],
                                 start=True, stop=True)
                if split_evac and nn % 2 == 0:
                    # ACT is idle at the kernel tail: split the evacuation
                    nc.scalar.activation(ob[:, nn * 512:(nn + 1) * 512], op,
                                         AF.Copy)
                else:
                    nc.vector.tensor_copy(ob[:, nn * 512:(nn + 1) * 512], op)
                yield 512 * 0.42
            # out-stores go via the idle Pool engine's SWDGE queue: their
            # data-ready waits must not block the in-order SP/HWDGE queue
            # that feeds latency-critical xw loads
            nc.gpsimd.dma_start(out=out[gq0:gq0 + P, :], in_=ob)

        # ---- attention chunks ----
        def emit_ctx(b, icn, jt, ex):
            d = jt - icn * QT
            need_until(uid_v[b * NJ + jt])
            if jt == 0:
                # new chunk touches every acc region: prior chunks' tail
                # reads must already be emitted (WAR via program order);
                # a little filler covers the DVE normalize latency so the
                # first ctx doesn't stall on the acc WAR
                flush_tails()
                for _ in range(BFILL):
                    if work:
                        pump_one()
            for h in range(2):
                for qs in range(max(d, 0), QT):
                    acc = accA if qs < 2 else accB
                    # one start=True per acc bank per chunk: start pends the
                    # whole 2KB zero region, so the other groups' first
                    # accumulate materializes zero+write (lazy per-byte).
                    # All writers are PE matmuls -> program order holds.
                    nc.tensor.matmul(
                        acc[:, qs % 2, h, :],
                        ex[:, h, qs * P:(qs + 1) * P],
                        vA[:, b * NJ + jt, h * 65:(h + 1) * 65],
                        start=(jt == 0 and h == 0 and qs % 2 == 0),
                        stop=(jt == icn * QT + qs),
                        skip_group_check=True)
                    est["pe"] += 65 * 0.42
            if d >= 0:
                last = (b == B - 1 and icn == NIC - 1)
                tails.append((est["iter"] + TAIL_LAG, tail1_unit(b, icn, d)))
                tails2.append((est["iter"] + T2_LAG,
                               tail2_unit(b, icn, d, split_evac=last)))

        # lag-2 software pipeline: ctx(jt-2) is emitted after scores(jt), so
        # PE never reaches a ctx matmul before its exp (ACT, ~1.07us after
        # its scores) has retired; pend carries across chunk boundaries
        # p-state warmup: the PE ramps to full clock only after 3us of
        # continuous execution; burn the initial xw-DMA wait on dummy
        # matmuls so the first projections run at full speed
        for i in range(4):
            wp = sm_ps.tile([P, 512], F32, name="wp", tag="sm")
            nc.tensor.matmul(wp, warm[:, 0:P], warm, start=True, stop=True)

        pend = deque()
        for b in range(B):
            for icn in range(NIC):
                # per-chunk meter reset: a cumulative surplus from the
                # PE-bound early phase must not block pre-draining filler
                # during ACT-bound chunks
                est["pe"] = est["act"] = 0.0
                gi0 = b * S + icn * IC
                njt = (icn + 1) * QT
                nw = (b * S + (icn + 1) * IC + WN - 1) // WN
                enqueue_window(nw + AHEAD)
                need_until(uid_q[(b * S + icn * IC) // WN])
                last_chunk = (b == B - 1 and icn == NIC - 1)
                for jt in range(njt):
                    est["iter"] += 1
                    if icn >= 5:
                        # ACT-bound chunk: filler first, so PE doesn't sit
                        # stalled on the sc-buffer WAR (exp jt-2) and reset
                        # its p-state ramp
                        pump_to_balance()
                    # scores+exp go out first each iteration so the ACT
                    # engine's exp queue never waits behind tail/filler work
                    need_until(uid_k[b * NJ + jt])
                    il0 = max(0, jt * P - icn * IC)
                    gj0 = b * S + jt * P
                    sc = sc_ps.tile([P, 2, IC], F32, name="sc", tag="sc")
                    for h in range(2):
                        hp = h * 64
                        nc.tensor.matmul(
                            sc[:, h, il0:IC],
                            kT[hp:hp + 64, gj0:gj0 + P],
                            qT[hp:hp + 64, gi0 + il0:gi0 + IC],
                            start=True, stop=True)
                    est["pe"] += 2 * (IC - il0) * 0.42
                    ex = ex_sb.tile([P, 2, IC], BF16, name="ex", tag="ex")
                    nc.scalar.activation(ex[:, :, il0:IC], sc[:, :, il0:IC],
                                         AF.Exp, scale=0.125)
                    est["act"] += 2 * (IC - il0) * 0.84 + 185
                    if jt * P >= icn * IC:  # diagonal tile
                        for h in range(2):
                            nc.vector.tensor_mul(ex[:, h, il0:il0 + P],
                                                 ex[:, h, il0:il0 + P],
                                                 mask_s)
                    if last_chunk:
                        # no later chunk will absorb the backlog: drain all
                        # ready tails now so the end tail stays short
                        drain_tails(4)
                        if jt >= njt - 8:
                            flush_tails()
                            _drain(tails2, len(tails2), True)
                    else:
                        drain_tails(NTAILS)
                    pend.append((b, icn, jt, ex))
                    # deeper ctx lag in short early chunks: gives the prev
                    # chunk's last tail1 normalize more slack before the
                    # first ctx of this chunk hits the acc WAR
                    plag = 3 if icn <= int(os.environ.get('KV2_PLAGC', '7')) else 2
                    if len(pend) > plag:
                        emit_ctx(*pend.popleft())
                    pump_to_balance()
                # drain the ctx pipeline at chunk end so the last tiles'
                # tail1 normalizes get a couple of iterations of slack
                # before the next chunk's first ctx (acc WAR); filler first
                # so the last ctx doesn't stall on its just-emitted exp
                for _ in range(BFILL):
                    if work:
                        pump_one()
                while pend:
                    emit_ctx(*pend.popleft())
        flush_tails()
        _drain(tails2, len(tails2), True)
        while work:
            pump_one()

    nc.compile()
    return nc


def _warrange(w, bf16):
    # [D, CW] -> [P, D//P, CW] contiguous (the SBUF layout, so the DMA is
    # a single contiguous copy instead of 256B strided pieces)
    D, CW_ = w.shape
    return np.ascontiguousarray(
        w.reshape(D // P, P, CW_).transpose(1, 0, 2)).astype(bf16)


def make_in_maps(x, Wq, Wk, Wv, Wo):
    import ml_dtypes
    bf16 = ml_dtypes.bfloat16
    B, S, D = x.shape
    xT = np.ascontiguousarray(x.reshape(B * S, D).T).astype(bf16)
    mask = np.triu(np.ones((P, P), dtype=bf16))
    ident = np.eye(P, dtype=bf16)
    in_maps = []
    for c in range(N_CORES):
        cs = slice(c * CW, (c + 1) * CW)
        in_maps.append({
            "xT": xT,
            "wq": _warrange(Wq[:, cs], bf16),
            "wk": _warrange(Wk[:, cs], bf16),
            "wv": _warrange(Wv[:, cs], bf16),
            "wo": np.ascontiguousarray(Wo[cs, :]).astype(bf16),
            "mask": mask,
            "ident": ident,
        })
    return in_maps


_CACHED_NC = None


def kernel(x, Wq, Wk, Wv, Wo, bo, _trace=False):
    global _CACHED_NC
    x = np.asarray(x, dtype=np.float32)
    B, S, D = x.shape
    if _CACHED_NC is None:
        _CACHED_NC = build_program(S=S, B=B, D=D)
    nc = _CACHED_NC
    in_maps = make_in_maps(x, np.asarray(Wq), np.asarray(Wk),
                           np.asarray(Wv), np.asarray(Wo))
    res = None
    for attempt in range(3):
        try:
            res = run_bass_kernel_spmd(nc, in_maps, list(range(N_CORES)),
                                       trace=_trace)
            break
        except Exception:
            if attempt == 2:
                raise
    out = np.zeros((B * S, D), dtype=np.float32)
    for c in range(N_CORES):
        out += res.results[c]["out"].astype(np.float32)
    out += np.asarray(bo, dtype=np.float32)[None, :]
    if _trace:
        kernel._last_result = res
    return out.reshape(B, S, D)


# revision 5
# speedup vs baseline: 1.2896x; 1.0016x over previous
"""Multi-head causal attention (B=2, S=4096, D=1024, H=16) on 8 TRN2 NeuronCores.

Sharding: head-parallel. Core c computes heads 2c, 2c+1 (128 of the 1024
projection columns) for both batches:
  - QKV column-parallel: each core gets Wq/Wk/Wv[:, c*128:(c+1)*128]
  - out-proj row-parallel: partial_out = ctx_c @ Wo[c*128:(c+1)*128, :]
  - host sums the 8 partials and adds bo.

The TimelineSim cost model charges a matmul only by its OUTPUT free size
(N columns), so the ctx product is blocked transposed: per (128-query x
128-key) causal block, out[128 q, 65] += ex[128 k, 128 q].T @ vA[128 k, 65]
streams just 65 columns (the 65th is an all-ones column producing the
softmax denominator), instead of re-streaming 512 query columns per key
tile. The denominator then sits on the free dim, so normalization is a
cheap per-partition tensor_scalar, and a single PE transpose per query
tile restores the [cdim, token] layout for the out-projection.

Layouts on-chip (per core):
  qT, kT:  [128, T]  rows 0:64 head0, 64:128 head1 (transposed projections)
  vA:      [128, T/128, 130]  per key-tile [v_h0 | ones | v_h1 | ones]
  sc/ex:   [128 keys, 2 heads, 512 queries]  PSUM scores / SBUF exp(bf16)
  acc:     PSUM [128 q, 2 qsub, 2 heads, 65] x2 (unnormalized ctx + denom)
  cT:      [128 cdim, T]  normalized ctx, transposed back per 128-q tile

Emission keeps both bottleneck engines (PE ~283us of matmul columns,
ACT ~280us of exp) near-continuously busy: QKV projection windows are
generators on a work queue, metered one matmul at a time between
attention iterations (forced just-in-time by fine-grained need-barriers,
pre-drained during ACT-bound chunks); per-tile tail work drains with an
iteration lag so its cross-engine latency chains stay off the PE stream;
out-stores ride the idle Pool engine's SWDGE queue so their data waits
never block the in-order SP queue feeding xw loads; deep SBUF pools
(ex/nrm/rcp/ob) keep WAR rotations off the critical path; dummy warm-up
matmuls ramp the PE p-state during the initial DMA fill.
"""

from collections import deque
from contextlib import ExitStack

import numpy as np

import concourse.bass as bass
import concourse.tile as tile
from concourse import bacc, mybir
from concourse.bass_utils import run_bass_kernel_spmd

F32 = mybir.dt.float32
BF16 = mybir.dt.bfloat16
P = 128
AF = mybir.ActivationFunctionType

N_CORES = 8
B_FULL, S_FULL, D_FULL, H_FULL = 2, 4096, 1024, 16
DH = 64
CW = 128  # projection columns per core (2 heads * 64)


def build_program(S=S_FULL, B=B_FULL, D=D_FULL):
    """Build the per-core Bass program (same program on all 8 cores)."""
    T = B * S
    KC = D // P            # contraction chunks for the projections
    IC = min(512, S)       # query-chunk width
    QT = IC // P           # query tiles per chunk (4)
    NJ = S // P            # key tiles per batch
    NIC = S // IC          # query chunks per batch
    WN = min(512, T)       # QKV token window

    nc = bacc.Bacc("TRN2", target_bir_lowering=False, debug=False,
                   num_devices=N_CORES)

    xT = nc.dram_tensor("xT", [D, T], BF16, kind="ExternalInput").ap()
    wq = nc.dram_tensor("wq", [P, D // P, CW], BF16, kind="ExternalInput").ap()
    wk = nc.dram_tensor("wk", [P, D // P, CW], BF16, kind="ExternalInput").ap()
    wv = nc.dram_tensor("wv", [P, D // P, CW], BF16, kind="ExternalInput").ap()
    wo = nc.dram_tensor("wo", [CW, D], BF16, kind="ExternalInput").ap()
    mask = nc.dram_tensor("mask", [P, P], BF16, kind="ExternalInput").ap()
    ident = nc.dram_tensor("ident", [P, P], BF16, kind="ExternalInput").ap()
    out = nc.dram_tensor("out", [T, D], BF16, kind="ExternalOutput").ap()

    with tile.TileContext(nc) as tc, ExitStack() as ctx:
        singles = ctx.enter_context(tc.tile_pool(name="singles", bufs=1))
        qT = singles.tile([P, T], BF16, name="qT")
        kT = singles.tile([P, T], BF16, name="kT")
        vA = singles.tile([P, B * NJ, 130], BF16, name="vA")
        cT = singles.tile([P, T], BF16, name="cT")
        wq_s = singles.tile([P, KC, CW], BF16, name="wq_s")
        wk_s = singles.tile([P, KC, CW], BF16, name="wk_s")
        wv_s = singles.tile([P, KC, CW], BF16, name="wv_s")
        wo_s = singles.tile([CW, D], BF16, name="wo_s")
        mask_s = singles.tile([P, P], BF16, name="mask_s")
        ident_s = singles.tile([P, P], BF16, name="ident_s")

        # weight loads dispatch on the ACT hwdge queue, in parallel with
        # the SP queue's first xw window loads
        nc.scalar.dma_start(out=wq_s, in_=wq)
        nc.scalar.dma_start(out=wk_s, in_=wk)
        nc.scalar.dma_start(out=wv_s, in_=wv)
        nc.vector.memset(vA[:, :, 64:65], 1.0)
        nc.vector.memset(vA[:, :, 129:130], 1.0)
        warm = singles.tile([P, 512], BF16, name="warm")
        nc.vector.memset(warm, 0.0)

        # PSUM budget (8 banks): sc 2x2 + acc 2x1 + shared-small 2x1 = 8
        ps_single = ctx.enter_context(
            tc.tile_pool(name="ps_single", bufs=1, space=bass.MemorySpace.PSUM))
        # [q, qsub-pair, head, v+denom]; accA holds qsub 0,1; accB 2,3
        accA = ps_single.tile([P, 2, 2, 65], F32, name="accA")
        accB = ps_single.tile([P, 2, 2, 65], F32, name="accB")
        sc_ps = ctx.enter_context(
            tc.tile_pool(name="sc_ps", bufs=2, space=bass.MemorySpace.PSUM))
        sm_ps = ctx.enter_context(
            tc.tile_pool(name="sm_ps", bufs=2, space=bass.MemorySpace.PSUM))
        xw_pool = ctx.enter_context(tc.tile_pool(name="xw_pool", bufs=10))
        ex_sb = ctx.enter_context(tc.tile_pool(name="ex_sb", bufs=8))
        nrm_sb = ctx.enter_context(tc.tile_pool(name="nrm_sb", bufs=8))
        rcp_sb = ctx.enter_context(tc.tile_pool(name="rcp_sb", bufs=8))
        ob_sb = ctx.enter_context(tc.tile_pool(name="ob_sb", bufs=4))

        # ---- deferred-work queue of generators yielding ~PE-ns steps ----
        # Filler (QKV windows) is metered one matmul at a time between
        # attention iterations so the ACT engine's exp queue never runs dry
        # and PE never stalls (a PE stall resets the p-state ramp to half
        # clock for 3us). Tail units (normalize/transpose/out-proj) are
        # latency chains: they drain with priority, max 2 per iteration.
        import os
        work = deque()   # entries: (uid, generator)
        tails = deque()  # entries: (ready_iter, generator)
        tails2 = deque()  # out-proj units: lazy, never forced at boundaries
        est = {"pe": 0.0, "act": 0.0, "uid": 0, "done": -1, "iter": 0}
        MARGIN = float(os.environ.get("KV2_MARGIN", "0"))
        TAIL_LAG = int(os.environ.get("KV2_TAIL_LAG", "4"))
        T2_LAG = int(os.environ.get("KV2_T2_LAG", "5"))
        AHEAD = int(os.environ.get("KV2_AHEAD", "4"))
        NTAILS = int(os.environ.get("KV2_NTAILS", "2"))
        BFILL = int(os.environ.get("KV2_BFILL", "0"))

        def pump_one():
            uid, gen = work[0]
            try:
                est["pe"] += next(gen)
            except StopIteration:
                work.popleft()
                est["done"] = uid

        def pump_to_balance():
            while work and est["pe"] < est["act"] + MARGIN:
                pump_one()
            if not work and est["pe"] < est["act"]:
                # queue ran dry: the PE deficit is past idle time, not
                # fillable later — don't let it trigger a future dump
                est["pe"] = est["act"]

        def need_until(uid):
            while est["done"] < uid:
                pump_one()

        def _drain(dq, n, force):
            for _ in range(min(n, len(dq))):
                if not force and dq[0][0] > est["iter"]:
                    return
                for cost in dq.popleft()[1]:
                    est["pe"] += cost

        def drain_tails(n, force=False):
            _drain(tails, n, force)
            _drain(tails2, n, force)

        def flush_tails():
            # only tail1 (acc readers) must precede a new chunk's ctx; tail2
            # (out-proj) is exempt and drains lazily off the diag hot region
            _drain(tails, len(tails), True)

        # ---- QKV projection windows ----
        def q_unit(w, xw):
            p_ps = sm_ps.tile([P, WN], F32, name="p_ps", tag="sm")
            for kc in range(KC):
                nc.tensor.matmul(p_ps, wq_s[:, kc, :], xw[:, kc, :],
                                 start=(kc == 0), stop=(kc == KC - 1))
                yield WN * 0.42
            nc.vector.tensor_copy(qT[:, w * WN:(w + 1) * WN], p_ps)
            yield 0.0

        def k_unit(w, st, xw):
            t0 = w * WN + st * P
            p_ps = sm_ps.tile([P, P], F32, name="kp", tag="sm")
            for kc in range(KC):
                nc.tensor.matmul(p_ps, wk_s[:, kc, :],
                                 xw[:, kc, st * P:(st + 1) * P],
                                 start=(kc == 0), stop=(kc == KC - 1))
                if kc % 2 == 1:
                    yield 2 * P * 0.42
            nc.vector.tensor_copy(kT[:, t0:t0 + P], p_ps)
            yield 0.0

        def v_unit(w, st, xw):
            jt = (w * WN) // P + st
            vp = sm_ps.tile([P, CW], F32, name="vp", tag="sm")
            for kc in range(KC):
                nc.tensor.matmul(vp, xw[:, kc, st * P:(st + 1) * P],
                                 wv_s[:, kc, :],
                                 start=(kc == 0), stop=(kc == KC - 1))
                if kc % 2 == 1:
                    yield 2 * CW * 0.42
            nc.vector.tensor_copy(vA[:, jt, 0:64], vp[:, 0:64])
            nc.vector.tensor_copy(vA[:, jt, 65:129], vp[:, 64:128])
            yield 0.0

        state = {"enqueued": 0}
        uid_q = {}
        uid_k = {}
        uid_v = {}

        def add_unit(gen):
            uid = est["uid"]
            est["uid"] += 1
            work.append((uid, gen))
            return uid

        def enqueue_window(upto):
            while state["enqueued"] < min(upto, T // WN):
                w = state["enqueued"]
                xw = xw_pool.tile([P, KC, WN], BF16, name="xw", tag="xw")
                # 2 kc-chunks per dma_start: halves SP dispatch cost while
                # still spreading the window across parallel DMA engines
                for kc in range(0, KC, 2):
                    src = bass.AP(tensor=xT.tensor, offset=kc * P * T + w * WN,
                                  ap=[[T, P], [P * T, 2], [1, WN]])
                    nc.sync.dma_start(out=xw[:, kc:kc + 2, :], in_=src)
                if w == 0:
                    # deferred: not needed before the first attention chunk
                    nc.scalar.dma_start(out=wo_s, in_=wo)
                    nc.scalar.dma_start(out=mask_s, in_=mask)
                    nc.scalar.dma_start(out=ident_s, in_=ident)
                uid_q[w] = add_unit(q_unit(w, xw))
                for st in range(WN // P):
                    gt = (w * WN) // P + st
                    uid_k[gt] = add_unit(k_unit(w, st, xw))
                    uid_v[gt] = add_unit(v_unit(w, st, xw))
                state["enqueued"] += 1

        # ---- per-query-tile tail: normalize + transpose + out-proj ----
        def tail1_unit(b, icn, qs):
            acc = accA if qs < 2 else accB
            i2 = qs % 2
            gq0 = b * S + (icn * QT + qs) * P
            rcp = rcp_sb.tile([P, 2, 1], F32, name="rcp", tag="rcp")
            nc.vector.reciprocal(rcp, acc[:, i2, :, 64:65])
            nrmd = nrm_sb.tile([P, P], BF16, name="nrmd", tag="nrmd")
            for h in range(2):
                nc.vector.tensor_scalar_mul(
                    nrmd[:, h * 64:(h + 1) * 64],
                    acc[:, i2, h, 0:64], rcp[:, h, :])
            yield 0.0
            tT = sm_ps.tile([P, P], BF16, name="tT", tag="sm")
            nc.tensor.transpose(tT, nrmd, ident_s)
            nc.vector.tensor_copy(cT[:, gq0:gq0 + P], tT)
            yield P * 0.42

        def tail2_unit(b, icn, qs, split_evac=False):
            gq0 = b * S + (icn * QT + qs) * P
            ob = ob_sb.tile([P, D], BF16, name="ob", tag="ob")
            for nn in range(D // 512):
                op = sm_ps.tile([P, 512], F32, name="op", tag="sm")
                nc.tensor.matmul(op, cT[:, gq0:gq0 + P],
                                 wo_s[:, nn * 512:(nn + 1) * 512],
                                 start=True, stop=True)
                if split_evac and nn % 2 == 0:
                    # ACT is idle at the kernel tail: split the evacuation
                    nc.scalar.activation(ob[:, nn * 512:(nn + 1) * 512], op,
                                         AF.Copy)
                else:
                    nc.vector.tensor_copy(ob[:, nn * 512:(nn + 1) * 512], op)
                yield 512 * 0.42
            # out-stores go via the idle Pool engine's SWDGE queue: their
            # data-ready waits must not block the in-order SP/HWDGE queue
            # that feeds latency-critical xw loads. Exception: the last
            # chunk's stores use the SP queue (empty by then) — the final
            # drain barrier otherwise waits ~1us of SWDGE descriptor
            # generation per store, serial on Pool
            if split_evac:
                nc.sync.dma_start(out=out[gq0:gq0 + P, :], in_=ob)
            else:
                nc.gpsimd.dma_start(out=out[gq0:gq0 + P, :], in_=ob)

        # ---- attention chunks ----
        def emit_ctx(b, icn, jt, ex):
            d = jt - icn * QT
            need_until(uid_v[b * NJ + jt])
            if jt == 0:
                # new chunk touches every acc region: prior chunks' tail
                # reads must already be emitted (WAR via program order);
                # a little filler covers the DVE normalize latency so the
                # first ctx doesn't stall on the acc WAR
                flush_tails()
                for _ in range(BFILL):
                    if work:
                        pump_one()
            for h in range(2):
                for qs in range(max(d, 0), QT):
                    acc = accA if qs < 2 else accB
                    # one start=True per acc bank per chunk: start pends the
                    # whole 2KB zero region, so the other groups' first
                    # accumulate materializes zero+write (lazy per-byte).
                    # All writers are PE matmuls -> program order holds.
                    nc.tensor.matmul(
                        acc[:, qs % 2, h, :],
                        ex[:, h, qs * P:(qs + 1) * P],
                        vA[:, b * NJ + jt, h * 65:(h + 1) * 65],
                        start=(jt == 0 and h == 0 and qs % 2 == 0),
                        stop=(jt == icn * QT + qs),
                        skip_group_check=True)
                    est["pe"] += 65 * 0.42
            if d >= 0:
                last = (b == B - 1 and icn == NIC - 1)
                tails.append((est["iter"] + TAIL_LAG, tail1_unit(b, icn, d)))
                tails2.append((est["iter"] + T2_LAG,
                               tail2_unit(b, icn, d, split_evac=last)))

        # lag-2 software pipeline: ctx(jt-2) is emitted after scores(jt), so
        # PE never reaches a ctx matmul before its exp (ACT, ~1.07us after
        # its scores) has retired; pend carries across chunk boundaries
        # p-state warmup: the PE ramps to full clock only after 3us of
        # continuous execution; burn the initial xw-DMA wait on dummy
        # matmuls so the first projections run at full speed
        for i in range(4):
            wp = sm_ps.tile([P, 512], F32, name="wp", tag="sm")
            nc.tensor.matmul(wp, warm[:, 0:P], warm, start=True, stop=True)

        pend = deque()
        for b in range(B):
            for icn in range(NIC):
                # per-chunk meter reset: a cumulative surplus from the
                # PE-bound early phase must not block pre-draining filler
                # during ACT-bound chunks
                est["pe"] = est["act"] = 0.0
                gi0 = b * S + icn * IC
                njt = (icn + 1) * QT
                nw = (b * S + (icn + 1) * IC + WN - 1) // WN
                enqueue_window(nw + AHEAD)
                need_until(uid_q[(b * S + icn * IC) // WN])
                last_chunk = (b == B - 1 and icn == NIC - 1)
                for jt in range(njt):
                    est["iter"] += 1
                    if icn >= 5:
                        # ACT-bound chunk: filler first, so PE doesn't sit
                        # stalled on the sc-buffer WAR (exp jt-2) and reset
                        # its p-state ramp
                        pump_to_balance()
                    # scores+exp go out first each iteration so the ACT
                    # engine's exp queue never waits behind tail/filler work
                    need_until(uid_k[b * NJ + jt])
                    il0 = max(0, jt * P - icn * IC)
                    gj0 = b * S + jt * P
                    sc = sc_ps.tile([P, 2, IC], F32, name="sc", tag="sc")
                    for h in range(2):
                        hp = h * 64
                        nc.tensor.matmul(
                            sc[:, h, il0:IC],
                            kT[hp:hp + 64, gj0:gj0 + P],
                            qT[hp:hp + 64, gi0 + il0:gi0 + IC],
                            start=True, stop=True)
                    est["pe"] += 2 * (IC - il0) * 0.42
                    ex = ex_sb.tile([P, 2, IC], BF16, name="ex", tag="ex")
                    nc.scalar.activation(ex[:, :, il0:IC], sc[:, :, il0:IC],
                                         AF.Exp, scale=0.125)
                    est["act"] += 2 * (IC - il0) * 0.84 + 185
                    if jt * P >= icn * IC:  # diagonal tile
                        for h in range(2):
                            nc.vector.tensor_mul(ex[:, h, il0:il0 + P],
                                                 ex[:, h, il0:il0 + P],
                                                 mask_s)
                    if last_chunk:
                        # no later chunk will absorb the backlog: drain all
                        # ready tails now so the end tail stays short
                        drain_tails(4)
                        if jt >= njt - 8:
                            flush_tails()
                            _drain(tails2, len(tails2), True)
                    else:
                        drain_tails(NTAILS)
                    pend.append((b, icn, jt, ex))
                    # deeper ctx lag in short early chunks: gives the prev
                    # chunk's last tail1 normalize more slack before the
                    # first ctx of this chunk hits the acc WAR
                    plag = 3 if icn <= int(os.environ.get('KV2_PLAGC', '7')) else 2
                    if len(pend) > plag:
                        emit_ctx(*pend.popleft())
                    pump_to_balance()
                # drain the ctx pipeline at chunk end so the last tiles'
                # tail1 normalizes get a couple of iterations of slack
                # before the next chunk's first ctx (acc WAR); filler first
                # so the last ctx doesn't stall on its just-emitted exp
                for _ in range(BFILL):
                    if work:
                        pump_one()
                while pend:
                    emit_ctx(*pend.popleft())
        flush_tails()
        _drain(tails2, len(tails2), True)
        while work:
            pump_one()

    nc.compile()
    return nc


def _warrange(w, bf16):
    # [D, CW] -> [P, D//P, CW] contiguous (the SBUF layout, so the DMA is
    # a single contiguous copy instead of 256B strided pieces)
    D, CW_ = w.shape
    return np.ascontiguousarray(
        w.reshape(D // P, P, CW_).transpose(1, 0, 2)).astype(bf16)


def make_in_maps(x, Wq, Wk, Wv, Wo):
    import ml_dtypes
    bf16 = ml_dtypes.bfloat16
    B, S, D = x.shape
    xT = np.ascontiguousarray(x.reshape(B * S, D).T).astype(bf16)
    mask = np.triu(np.ones((P, P), dtype=bf16))
    ident = np.eye(P, dtype=bf16)
    in_maps = []
    for c in range(N_CORES):
        cs = slice(c * CW, (c + 1) * CW)
        in_maps.append({
            "xT": xT,
            "wq": _warrange(Wq[:, cs], bf16),
            "wk": _warrange(Wk[:, cs], bf16),
            "wv": _warrange(Wv[:, cs], bf16),
            "wo": np.ascontiguousarray(Wo[cs, :]).astype(bf16),
            "mask": mask,
            "ident": ident,
        })
    return in_maps


_CACHED_NC = None


def kernel(x, Wq, Wk, Wv, Wo, bo, _trace=False):
    global _CACHED_NC
    x = np.asarray(x, dtype=np.float32)
    B, S, D = x.shape
    if _CACHED_NC is None:
        _CACHED_NC = build_program(S=S, B=B, D=D)
    nc = _CACHED_NC
    in_maps = make_in_maps(x, np.asarray(Wq), np.asarray(Wk),
                           np.asarray(Wv), np.asarray(Wo))
    res = None
    for attempt in range(3):
        try:
            res = run_bass_kernel_spmd(nc, in_maps, list(range(N_CORES)),
                                       trace=_trace)
            break
        except Exception:
            if attempt == 2:
                raise
    out = np.zeros((B * S, D), dtype=np.float32)
    for c in range(N_CORES):
        out += res.results[c]["out"].astype(np.float32)
    out += np.asarray(bo, dtype=np.float32)[None, :]
    if _trace:
        kernel._last_result = res
    return out.reshape(B, S, D)


# revision 6
# speedup vs baseline: 1.2898x; 1.0002x over previous
"""Multi-head causal attention (B=2, S=4096, D=1024, H=16) on 8 TRN2 NeuronCores.

Sharding: head-parallel. Core c computes heads 2c, 2c+1 (128 of the 1024
projection columns) for both batches:
  - QKV column-parallel: each core gets Wq/Wk/Wv[:, c*128:(c+1)*128]
  - out-proj row-parallel: partial_out = ctx_c @ Wo[c*128:(c+1)*128, :]
  - host sums the 8 partials and adds bo.

The TimelineSim cost model charges a matmul only by its OUTPUT free size
(N columns), so the ctx product is blocked transposed: per (128-query x
128-key) causal block, out[128 q, 65] += ex[128 k, 128 q].T @ vA[128 k, 65]
streams just 65 columns (the 65th is an all-ones column producing the
softmax denominator), instead of re-streaming 512 query columns per key
tile. The denominator then sits on the free dim, so normalization is a
cheap per-partition tensor_scalar, and a single PE transpose per query
tile restores the [cdim, token] layout for the out-projection.

Layouts on-chip (per core):
  qT, kT:  [128, T]  rows 0:64 head0, 64:128 head1 (transposed projections)
  vA:      [128, T/128, 130]  per key-tile [v_h0 | ones | v_h1 | ones]
  sc/ex:   [128 keys, 2 heads, 512 queries]  PSUM scores / SBUF exp(bf16)
  acc:     PSUM [128 q, 2 qsub, 2 heads, 65] x2 (unnormalized ctx + denom)
  cT:      [128 cdim, T]  normalized ctx, transposed back per 128-q tile

Emission keeps both bottleneck engines (PE ~283us of matmul columns,
ACT ~280us of exp) near-continuously busy: QKV projection windows are
generators on a work queue, metered one matmul at a time between
attention iterations (forced just-in-time by fine-grained need-barriers,
pre-drained during ACT-bound chunks); per-tile tail work drains with an
iteration lag so its cross-engine latency chains stay off the PE stream;
out-stores ride the idle Pool engine's SWDGE queue so their data waits
never block the in-order SP queue feeding xw loads; deep SBUF pools
(ex/nrm/rcp/ob) keep WAR rotations off the critical path; dummy warm-up
matmuls ramp the PE p-state during the initial DMA fill.
"""

from collections import deque
from contextlib import ExitStack

import numpy as np

import concourse.bass as bass
import concourse.tile as tile
from concourse import bacc, mybir
from concourse.bass_utils import run_bass_kernel_spmd

F32 = mybir.dt.float32
BF16 = mybir.dt.bfloat16
P = 128
AF = mybir.ActivationFunctionType

N_CORES = 8
B_FULL, S_FULL, D_FULL, H_FULL = 2, 4096, 1024, 16
DH = 64
CW = 128  # projection columns per core (2 heads * 64)


def build_program(S=S_FULL, B=B_FULL, D=D_FULL):
    """Build the per-core Bass program (same program on all 8 cores)."""
    T = B * S
    KC = D // P            # contraction chunks for the projections
    IC = min(512, S)       # query-chunk width
    QT = IC // P           # query tiles per chunk (4)
    NJ = S // P            # key tiles per batch
    NIC = S // IC          # query chunks per batch
    WN = min(512, T)       # QKV token window

    nc = bacc.Bacc("TRN2", target_bir_lowering=False, debug=False,
                   num_devices=N_CORES)

    xT = nc.dram_tensor("xT", [D, T], BF16, kind="ExternalInput").ap()
    wq = nc.dram_tensor("wq", [P, D // P, CW], BF16, kind="ExternalInput").ap()
    wk = nc.dram_tensor("wk", [P, D // P, CW], BF16, kind="ExternalInput").ap()
    wv = nc.dram_tensor("wv", [P, D // P, CW], BF16, kind="ExternalInput").ap()
    wo = nc.dram_tensor("wo", [CW, D], BF16, kind="ExternalInput").ap()
    mask = nc.dram_tensor("mask", [P, P], BF16, kind="ExternalInput").ap()
    ident = nc.dram_tensor("ident", [P, P], BF16, kind="ExternalInput").ap()
    out = nc.dram_tensor("out", [T, D], BF16, kind="ExternalOutput").ap()

    with tile.TileContext(nc) as tc, ExitStack() as ctx:
        singles = ctx.enter_context(tc.tile_pool(name="singles", bufs=1))
        qT = singles.tile([P, T], BF16, name="qT")
        kT = singles.tile([P, T], BF16, name="kT")
        vA = singles.tile([P, B * NJ, 130], BF16, name="vA")
        cT = singles.tile([P, T], BF16, name="cT")
        wq_s = singles.tile([P, KC, CW], BF16, name="wq_s")
        wk_s = singles.tile([P, KC, CW], BF16, name="wk_s")
        wv_s = singles.tile([P, KC, CW], BF16, name="wv_s")
        wo_s = singles.tile([CW, D], BF16, name="wo_s")
        mask_s = singles.tile([P, P], BF16, name="mask_s")
        ident_s = singles.tile([P, P], BF16, name="ident_s")

        # weight loads dispatch on the ACT hwdge queue, in parallel with
        # the SP queue's first xw window loads
        nc.scalar.dma_start(out=wq_s, in_=wq)
        nc.scalar.dma_start(out=wk_s, in_=wk)
        nc.scalar.dma_start(out=wv_s, in_=wv)
        nc.vector.memset(vA[:, :, 64:65], 1.0)
        nc.vector.memset(vA[:, :, 129:130], 1.0)
        warm = singles.tile([P, 512], BF16, name="warm")
        nc.vector.memset(warm, 0.0)

        # PSUM budget (8 banks): sc 2x2 + acc 2x1 + shared-small 2x1 = 8
        ps_single = ctx.enter_context(
            tc.tile_pool(name="ps_single", bufs=1, space=bass.MemorySpace.PSUM))
        # [q, qsub-pair, head, v+denom]; accA holds qsub 0,1; accB 2,3
        accA = ps_single.tile([P, 2, 2, 65], F32, name="accA")
        accB = ps_single.tile([P, 2, 2, 65], F32, name="accB")
        sc_ps = ctx.enter_context(
            tc.tile_pool(name="sc_ps", bufs=2, space=bass.MemorySpace.PSUM))
        sm_ps = ctx.enter_context(
            tc.tile_pool(name="sm_ps", bufs=2, space=bass.MemorySpace.PSUM))
        xw_pool = ctx.enter_context(tc.tile_pool(name="xw_pool", bufs=10))
        ex_sb = ctx.enter_context(tc.tile_pool(name="ex_sb", bufs=8))
        nrm_sb = ctx.enter_context(tc.tile_pool(name="nrm_sb", bufs=8))
        rcp_sb = ctx.enter_context(tc.tile_pool(name="rcp_sb", bufs=8))
        ob_sb = ctx.enter_context(tc.tile_pool(name="ob_sb", bufs=4))

        # ---- deferred-work queue of generators yielding ~PE-ns steps ----
        # Filler (QKV windows) is metered one matmul at a time between
        # attention iterations so the ACT engine's exp queue never runs dry
        # and PE never stalls (a PE stall resets the p-state ramp to half
        # clock for 3us). Tail units (normalize/transpose/out-proj) are
        # latency chains: they drain with priority, max 2 per iteration.
        import os
        work = deque()   # entries: (uid, generator)
        tails = deque()  # entries: (ready_iter, generator)
        tails2 = deque()  # out-proj units: lazy, never forced at boundaries
        est = {"pe": 0.0, "act": 0.0, "uid": 0, "done": -1, "iter": 0}
        MARGIN = float(os.environ.get("KV2_MARGIN", "0"))
        TAIL_LAG = int(os.environ.get("KV2_TAIL_LAG", "4"))
        T2_LAG = int(os.environ.get("KV2_T2_LAG", "5"))
        AHEAD = int(os.environ.get("KV2_AHEAD", "4"))
        NTAILS = int(os.environ.get("KV2_NTAILS", "2"))
        BFILL = int(os.environ.get("KV2_BFILL", "0"))

        def pump_one():
            uid, gen = work[0]
            try:
                est["pe"] += next(gen)
            except StopIteration:
                work.popleft()
                est["done"] = uid

        def pump_to_balance():
            while work and est["pe"] < est["act"] + MARGIN:
                pump_one()
            if not work and est["pe"] < est["act"]:
                # queue ran dry: the PE deficit is past idle time, not
                # fillable later — don't let it trigger a future dump
                est["pe"] = est["act"]

        def need_until(uid):
            while est["done"] < uid:
                pump_one()

        def _drain(dq, n, force):
            for _ in range(min(n, len(dq))):
                if not force and dq[0][0] > est["iter"]:
                    return
                for cost in dq.popleft()[1]:
                    est["pe"] += cost

        def drain_tails(n, force=False):
            _drain(tails, n, force)
            _drain(tails2, n, force)

        def flush_tails():
            # only tail1 (acc readers) must precede a new chunk's ctx; tail2
            # (out-proj) is exempt and drains lazily off the diag hot region
            _drain(tails, len(tails), True)

        # ---- QKV projection windows ----
        def q_unit(w, xw):
            p_ps = sm_ps.tile([P, WN], F32, name="p_ps", tag="sm")
            for kc in range(KC):
                nc.tensor.matmul(p_ps, wq_s[:, kc, :], xw[:, kc, :],
                                 start=(kc == 0), stop=(kc == KC - 1))
                yield WN * 0.42
            nc.vector.tensor_copy(qT[:, w * WN:(w + 1) * WN], p_ps)
            yield 0.0

        def k_unit(w, st, xw):
            t0 = w * WN + st * P
            p_ps = sm_ps.tile([P, P], F32, name="kp", tag="sm")
            for kc in range(KC):
                nc.tensor.matmul(p_ps, wk_s[:, kc, :],
                                 xw[:, kc, st * P:(st + 1) * P],
                                 start=(kc == 0), stop=(kc == KC - 1))
                if kc % 2 == 1:
                    yield 2 * P * 0.42
            nc.vector.tensor_copy(kT[:, t0:t0 + P], p_ps)
            yield 0.0

        def v_unit(w, st, xw):
            jt = (w * WN) // P + st
            vp = sm_ps.tile([P, CW], F32, name="vp", tag="sm")
            for kc in range(KC):
                nc.tensor.matmul(vp, xw[:, kc, st * P:(st + 1) * P],
                                 wv_s[:, kc, :],
                                 start=(kc == 0), stop=(kc == KC - 1))
                if kc % 2 == 1:
                    yield 2 * CW * 0.42
            nc.vector.tensor_copy(vA[:, jt, 0:64], vp[:, 0:64])
            nc.vector.tensor_copy(vA[:, jt, 65:129], vp[:, 64:128])
            yield 0.0

        state = {"enqueued": 0}
        uid_q = {}
        uid_k = {}
        uid_v = {}

        def add_unit(gen):
            uid = est["uid"]
            est["uid"] += 1
            work.append((uid, gen))
            return uid

        def enqueue_window(upto):
            while state["enqueued"] < min(upto, T // WN):
                w = state["enqueued"]
                xw = xw_pool.tile([P, KC, WN], BF16, name="xw", tag="xw")
                # 2 kc-chunks per dma_start: halves SP dispatch cost while
                # still spreading the window across parallel DMA engines
                for kc in range(0, KC, 2):
                    src = bass.AP(tensor=xT.tensor, offset=kc * P * T + w * WN,
                                  ap=[[T, P], [P * T, 2], [1, WN]])
                    nc.sync.dma_start(out=xw[:, kc:kc + 2, :], in_=src)
                if w == 0:
                    # deferred: not needed before the first attention chunk
                    nc.scalar.dma_start(out=wo_s, in_=wo)
                    nc.scalar.dma_start(out=mask_s, in_=mask)
                    nc.scalar.dma_start(out=ident_s, in_=ident)
                uid_q[w] = add_unit(q_unit(w, xw))
                for st in range(WN // P):
                    gt = (w * WN) // P + st
                    uid_k[gt] = add_unit(k_unit(w, st, xw))
                    uid_v[gt] = add_unit(v_unit(w, st, xw))
                state["enqueued"] += 1

        # ---- per-query-tile tail: normalize + transpose + out-proj ----
        def tail1_unit(b, icn, qs):
            acc = accA if qs < 2 else accB
            i2 = qs % 2
            gq0 = b * S + (icn * QT + qs) * P
            rcp = rcp_sb.tile([P, 2, 1], F32, name="rcp", tag="rcp")
            nc.vector.reciprocal(rcp, acc[:, i2, :, 64:65])
            nrmd = nrm_sb.tile([P, P], BF16, name="nrmd", tag="nrmd")
            for h in range(2):
                nc.vector.tensor_scalar_mul(
                    nrmd[:, h * 64:(h + 1) * 64],
                    acc[:, i2, h, 0:64], rcp[:, h, :])
            yield 0.0
            tT = sm_ps.tile([P, P], BF16, name="tT", tag="sm")
            nc.tensor.transpose(tT, nrmd, ident_s)
            nc.vector.tensor_copy(cT[:, gq0:gq0 + P], tT)
            yield P * 0.42

        def tail2_unit(b, icn, qs, split_evac=False):
            gq0 = b * S + (icn * QT + qs) * P
            ob = ob_sb.tile([P, D], BF16, name="ob", tag="ob")
            for nn in range(D // 512):
                op = sm_ps.tile([P, 512], F32, name="op", tag="sm")
                nc.tensor.matmul(op, cT[:, gq0:gq0 + P],
                                 wo_s[:, nn * 512:(nn + 1) * 512],
                                 start=True, stop=True)
                if split_evac and nn % 2 == 0:
                    # ACT is idle at the kernel tail: split the evacuation
                    nc.scalar.activation(ob[:, nn * 512:(nn + 1) * 512], op,
                                         AF.Copy)
                else:
                    nc.vector.tensor_copy(ob[:, nn * 512:(nn + 1) * 512], op)
                yield 512 * 0.42
            # out-stores go via the idle Pool engine's SWDGE queue: their
            # data-ready waits must not block the in-order SP/HWDGE queue
            # that feeds latency-critical xw loads. Exception: the last
            # chunk's stores use the SP queue (empty by then) — the final
            # drain barrier otherwise waits ~1us of SWDGE descriptor
            # generation per store, serial on Pool
            if split_evac:
                nc.sync.dma_start(out=out[gq0:gq0 + P, :], in_=ob)
            else:
                nc.gpsimd.dma_start(out=out[gq0:gq0 + P, :], in_=ob)

        # ---- attention chunks ----
        def emit_ctx(b, icn, jt, ex):
            d = jt - icn * QT
            need_until(uid_v[b * NJ + jt])
            if jt == 0:
                # new chunk touches every acc region: prior chunks' tail
                # reads must already be emitted (WAR via program order);
                # a little filler covers the DVE normalize latency so the
                # first ctx doesn't stall on the acc WAR
                flush_tails()
                for _ in range(BFILL):
                    if work:
                        pump_one()
            for h in range(2):
                for qs in range(max(d, 0), QT):
                    acc = accA if qs < 2 else accB
                    # one start=True per acc bank per chunk: start pends the
                    # whole 2KB zero region, so the other groups' first
                    # accumulate materializes zero+write (lazy per-byte).
                    # All writers are PE matmuls -> program order holds.
                    nc.tensor.matmul(
                        acc[:, qs % 2, h, :],
                        ex[:, h, qs * P:(qs + 1) * P],
                        vA[:, b * NJ + jt, h * 65:(h + 1) * 65],
                        start=(jt == 0 and h == 0 and qs % 2 == 0),
                        stop=(jt == icn * QT + qs),
                        skip_group_check=True)
                    est["pe"] += 65 * 0.42
            if d >= 0:
                last = (b == B - 1 and icn == NIC - 1)
                tails.append((est["iter"] + TAIL_LAG, tail1_unit(b, icn, d)))
                tails2.append((est["iter"] + T2_LAG,
                               tail2_unit(b, icn, d, split_evac=last)))

        # lag-2 software pipeline: ctx(jt-2) is emitted after scores(jt), so
        # PE never reaches a ctx matmul before its exp (ACT, ~1.07us after
        # its scores) has retired; pend carries across chunk boundaries
        # p-state warmup: the PE ramps to full clock only after 3us of
        # continuous execution; burn the initial xw-DMA wait on dummy
        # matmuls so the first projections run at full speed
        for i in range(4):
            wp = sm_ps.tile([P, 512], F32, name="wp", tag="sm")
            nc.tensor.matmul(wp, warm[:, 0:P], warm, start=True, stop=True)

        pend = deque()
        for b in range(B):
            for icn in range(NIC):
                # per-chunk meter reset: a cumulative surplus from the
                # PE-bound early phase must not block pre-draining filler
                # during ACT-bound chunks
                est["pe"] = est["act"] = 0.0
                gi0 = b * S + icn * IC
                njt = (icn + 1) * QT
                nw = (b * S + (icn + 1) * IC + WN - 1) // WN
                enqueue_window(nw + AHEAD)
                need_until(uid_q[(b * S + icn * IC) // WN])
                last_chunk = (b == B - 1 and icn == NIC - 1)
                for jt in range(njt):
                    est["iter"] += 1
                    if icn >= 5:
                        # ACT-bound chunk: filler first, so PE doesn't sit
                        # stalled on the sc-buffer WAR (exp jt-2) and reset
                        # its p-state ramp
                        pump_to_balance()
                    # scores+exp go out first each iteration so the ACT
                    # engine's exp queue never waits behind tail/filler work
                    need_until(uid_k[b * NJ + jt])
                    il0 = max(0, jt * P - icn * IC)
                    gj0 = b * S + jt * P
                    sc = sc_ps.tile([P, 2, IC], F32, name="sc", tag="sc")
                    for h in range(2):
                        hp = h * 64
                        nc.tensor.matmul(
                            sc[:, h, il0:IC],
                            kT[hp:hp + 64, gj0:gj0 + P],
                            qT[hp:hp + 64, gi0 + il0:gi0 + IC],
                            start=True, stop=True)
                    est["pe"] += 2 * (IC - il0) * 0.42
                    ex = ex_sb.tile([P, 2, IC], BF16, name="ex", tag="ex")
                    nc.scalar.activation(ex[:, :, il0:IC], sc[:, :, il0:IC],
                                         AF.Exp, scale=0.125)
                    est["act"] += 2 * (IC - il0) * 0.84 + 185
                    if jt * P >= icn * IC:  # diagonal tile
                        for h in range(2):
                            nc.vector.tensor_mul(ex[:, h, il0:il0 + P],
                                                 ex[:, h, il0:il0 + P],
                                                 mask_s)
                    if last_chunk:
                        # no later chunk will absorb the backlog: drain all
                        # ready tails now so the end tail stays short
                        drain_tails(4)
                        if jt >= njt - 8:
                            flush_tails()
                            _drain(tails2, len(tails2), True)
                    else:
                        drain_tails(NTAILS)
                    if jt == njt - 2 and not last_chunk:
                        # flush the next chunk's q-window now, while PE is
                        # hot (full p-state) and the diag region's ACT load
                        # is light — the same flush after the boundary stall
                        # would run at half clock
                        b2, icn2 = (b, icn + 1) if icn + 1 < NIC else (b + 1, 0)
                        need_until(uid_q[(b2 * S + icn2 * IC) // WN])
                        need_until(uid_k[b2 * NJ])
                        need_until(uid_v[b2 * NJ])
                    pend.append((b, icn, jt, ex))
                    # deeper ctx lag in short early chunks: gives the prev
                    # chunk's last tail1 normalize more slack before the
                    # first ctx of this chunk hits the acc WAR
                    plag = 3 if icn <= int(os.environ.get('KV2_PLAGC', '7')) else 2
                    if len(pend) > plag:
                        emit_ctx(*pend.popleft())
                    pump_to_balance()
                # drain the ctx pipeline at chunk end so the last tiles'
                # tail1 normalizes get a couple of iterations of slack
                # before the next chunk's first ctx (acc WAR); filler first
                # so the last ctx doesn't stall on its just-emitted exp
                for _ in range(BFILL):
                    if work:
                        pump_one()
                while pend:
                    emit_ctx(*pend.popleft())
        flush_tails()
        _drain(tails2, len(tails2), True)
        while work:
            pump_one()

    nc.compile()
    return nc


def _warrange(w, bf16):
    # [D, CW] -> [P, D//P, CW] contiguous (the SBUF layout, so the DMA is
    # a single contiguous copy instead of 256B strided pieces)
    D, CW_ = w.shape
    return np.ascontiguousarray(
        w.reshape(D // P, P, CW_).transpose(1, 0, 2)).astype(bf16)


def make_in_maps(x, Wq, Wk, Wv, Wo):
    import ml_dtypes
    bf16 = ml_dtypes.bfloat16
    B, S, D = x.shape
    xT = np.ascontiguousarray(x.reshape(B * S, D).T).astype(bf16)
    mask = np.triu(np.ones((P, P), dtype=bf16))
    ident = np.eye(P, dtype=bf16)
    in_maps = []
    for c in range(N_CORES):
        cs = slice(c * CW, (c + 1) * CW)
        in_maps.append({
            "xT": xT,
            "wq": _warrange(Wq[:, cs], bf16),
            "wk": _warrange(Wk[:, cs], bf16),
            "wv": _warrange(Wv[:, cs], bf16),
            "wo": np.ascontiguousarray(Wo[cs, :]).astype(bf16),
            "mask": mask,
            "ident": ident,
        })
    return in_maps


_CACHED_NC = None


def kernel(x, Wq, Wk, Wv, Wo, bo, _trace=False):
    global _CACHED_NC
    x = np.asarray(x, dtype=np.float32)
    B, S, D = x.shape
    if _CACHED_NC is None:
        _CACHED_NC = build_program(S=S, B=B, D=D)
    nc = _CACHED_NC
    in_maps = make_in_maps(x, np.asarray(Wq), np.asarray(Wk),
                           np.asarray(Wv), np.asarray(Wo))
    res = None
    for attempt in range(3):
        try:
            res = run_bass_kernel_spmd(nc, in_maps, list(range(N_CORES)),
                                       trace=_trace)
            break
        except Exception:
            if attempt == 2:
                raise
    out = np.zeros((B * S, D), dtype=np.float32)
    for c in range(N_CORES):
        out += res.results[c]["out"].astype(np.float32)
    out += np.asarray(bo, dtype=np.float32)[None, :]
    if _trace:
        kernel._last_result = res
    return out.reshape(B, S, D)


# revision 7
# speedup vs baseline: 1.3072x; 1.0135x over previous
"""Multi-head causal attention (B=2, S=4096, D=1024, H=16) on 8 TRN2 NeuronCores.

Sharding: head-parallel. Core c computes heads 2c, 2c+1 (128 of the 1024
projection columns) for both batches:
  - QKV column-parallel: each core gets Wq/Wk/Wv[:, c*128:(c+1)*128]
  - out-proj row-parallel: partial_out = ctx_c @ Wo[c*128:(c+1)*128, :]
  - host sums the 8 partials and adds bo.

The TimelineSim cost model charges a matmul only by its OUTPUT free size
(N columns), so the ctx product is blocked transposed: per (128-query x
128-key) causal block, out[128 q, 65] += ex[128 k, 128 q].T @ vA[128 k, 65]
streams just 65 columns (the 65th is an all-ones column producing the
softmax denominator), instead of re-streaming 512 query columns per key
tile. The denominator then sits on the free dim, so normalization is a
cheap per-partition tensor_scalar, and a single PE transpose per query
tile restores the [cdim, token] layout for the out-projection.

Layouts on-chip (per core):
  qT, kT:  [128, T]  rows 0:64 head0, 64:128 head1 (transposed projections)
  vA:      [128, T/128, 130]  per key-tile [v_h0 | ones | v_h1 | ones]
  sc/ex:   [128 keys, 2 heads, 512 queries]  PSUM scores / SBUF exp(bf16)
  acc:     PSUM [128 q, 2 qsub, 2 heads, 65] x2 (unnormalized ctx + denom)
  cT:      [128 cdim, T]  normalized ctx, transposed back per 128-q tile

Emission keeps both bottleneck engines (PE ~283us of matmul columns,
ACT ~280us of exp) near-continuously busy: QKV projection windows are
generators on a work queue, metered one matmul at a time between
attention iterations (forced just-in-time by fine-grained need-barriers,
pre-drained during ACT-bound chunks); per-tile tail work drains with an
iteration lag so its cross-engine latency chains stay off the PE stream;
out-stores ride the idle Pool engine's SWDGE queue so their data waits
never block the in-order SP queue feeding xw loads; deep SBUF pools
(ex/nrm/rcp/ob) keep WAR rotations off the critical path; dummy warm-up
matmuls ramp the PE p-state during the initial DMA fill.
"""

import os
from collections import deque
from contextlib import ExitStack

import numpy as np

import concourse.bass as bass
import concourse.tile as tile
from concourse import bacc, mybir
from concourse.bass_utils import run_bass_kernel_spmd

F32 = mybir.dt.float32
BF16 = mybir.dt.bfloat16
P = 128
AF = mybir.ActivationFunctionType

N_CORES = 8
B_FULL, S_FULL, D_FULL, H_FULL = 2, 4096, 1024, 16
DH = 64
CW = 128  # projection columns per core (2 heads * 64)


def build_program(S=S_FULL, B=B_FULL, D=D_FULL):
    """Build the per-core Bass program (same program on all 8 cores)."""
    T = B * S
    KC = D // P            # contraction chunks for the projections
    IC = min(512, S)       # query-chunk width
    QT = IC // P           # query tiles per chunk (4)
    NJ = S // P            # key tiles per batch
    NIC = S // IC          # query chunks per batch
    WN = min(512, T)       # QKV token window

    nc = bacc.Bacc("TRN2", target_bir_lowering=False, debug=False,
                   num_devices=N_CORES)

    xT = nc.dram_tensor("xT", [D, T], BF16, kind="ExternalInput").ap()
    wq = nc.dram_tensor("wq", [P, D // P, CW], BF16, kind="ExternalInput").ap()
    wk = nc.dram_tensor("wk", [P, D // P, CW], BF16, kind="ExternalInput").ap()
    wv = nc.dram_tensor("wv", [P, D // P, CW], BF16, kind="ExternalInput").ap()
    wo = nc.dram_tensor("wo", [CW, D], BF16, kind="ExternalInput").ap()
    mask = nc.dram_tensor("mask", [P, P], BF16, kind="ExternalInput").ap()
    ident = nc.dram_tensor("ident", [P, P], BF16, kind="ExternalInput").ap()
    out = nc.dram_tensor("out", [T, D], BF16, kind="ExternalOutput").ap()

    with tile.TileContext(nc) as tc, ExitStack() as ctx:
        singles = ctx.enter_context(tc.tile_pool(name="singles", bufs=1))
        qT = singles.tile([P, T], BF16, name="qT")
        kT = singles.tile([P, T], BF16, name="kT")
        vA = singles.tile([P, B * NJ, 130], BF16, name="vA")
        cT = singles.tile([P, T], BF16, name="cT")
        wq_s = singles.tile([P, KC, CW], BF16, name="wq_s")
        wk_s = singles.tile([P, KC, CW], BF16, name="wk_s")
        wv_s = singles.tile([P, KC, CW], BF16, name="wv_s")
        wo_s = singles.tile([CW, D], BF16, name="wo_s")
        mask_s = singles.tile([P, P], BF16, name="mask_s")
        ident_s = singles.tile([P, P], BF16, name="ident_s")

        # weight loads dispatch on the ACT hwdge queue, in parallel with
        # the SP queue's first xw window loads
        nc.scalar.dma_start(out=wq_s, in_=wq)
        nc.scalar.dma_start(out=wk_s, in_=wk)
        nc.scalar.dma_start(out=wv_s, in_=wv)
        nc.vector.memset(vA[:, :, 64:65], 1.0)
        nc.vector.memset(vA[:, :, 129:130], 1.0)
        warm = singles.tile([P, 512], BF16, name="warm")
        nc.vector.memset(warm, 0.0)

        # PSUM budget (8 banks): sc 2x2 + acc 2x1 + shared-small 2x1 = 8
        ps_single = ctx.enter_context(
            tc.tile_pool(name="ps_single", bufs=1, space=bass.MemorySpace.PSUM))
        # [q, qsub-pair, head, v+denom]; accA holds qsub 0,1; accB 2,3
        accA = ps_single.tile([P, 2, 2, 65], F32, name="accA")
        accB = ps_single.tile([P, 2, 2, 65], F32, name="accB")
        sc_ps = ctx.enter_context(
            tc.tile_pool(name="sc_ps", bufs=2, space=bass.MemorySpace.PSUM))
        sm_ps = ctx.enter_context(
            tc.tile_pool(name="sm_ps", bufs=2, space=bass.MemorySpace.PSUM))
        xw_pool = ctx.enter_context(tc.tile_pool(name="xw_pool", bufs=10))
        ex_sb = ctx.enter_context(tc.tile_pool(name="ex_sb", bufs=8))
        nrm_sb = ctx.enter_context(tc.tile_pool(name="nrm_sb", bufs=8))
        rcp_sb = ctx.enter_context(tc.tile_pool(name="rcp_sb", bufs=8))
        ob_sb = ctx.enter_context(tc.tile_pool(name="ob_sb", bufs=4))

        # ---- deferred-work queue of generators yielding ~PE-ns steps ----
        # Filler (QKV windows) is metered one matmul at a time between
        # attention iterations so the ACT engine's exp queue never runs dry
        # and PE never stalls (a PE stall resets the p-state ramp to half
        # clock for 3us). Tail units (normalize/transpose/out-proj) are
        # latency chains: they drain with priority, max 2 per iteration.
        import os
        work = deque()   # entries: (uid, generator)
        tails = deque()  # entries: (ready_iter, generator)
        tails2 = deque()  # out-proj units: lazy, never forced at boundaries
        est = {"pe": 0.0, "act": 0.0, "uid": 0, "done": -1, "iter": 0}
        MARGIN = float(os.environ.get("KV2_MARGIN", "0"))
        TAIL_LAG = int(os.environ.get("KV2_TAIL_LAG", "4"))
        T2_LAG = int(os.environ.get("KV2_T2_LAG", "5"))
        AHEAD = int(os.environ.get("KV2_AHEAD", "4"))
        NTAILS = int(os.environ.get("KV2_NTAILS", "2"))
        BFILL = int(os.environ.get("KV2_BFILL", "0"))

        def pump_one():
            uid, gen = work[0]
            try:
                est["pe"] += next(gen)
            except StopIteration:
                work.popleft()
                est["done"] = uid

        def pump_to_balance():
            while work and est["pe"] < est["act"] + MARGIN:
                pump_one()
            if not work and est["pe"] < est["act"]:
                # queue ran dry: the PE deficit is past idle time, not
                # fillable later — don't let it trigger a future dump
                est["pe"] = est["act"]

        def need_until(uid):
            while est["done"] < uid:
                pump_one()

        def _drain(dq, n, force):
            for _ in range(min(n, len(dq))):
                if not force and dq[0][0] > est["iter"]:
                    return
                for cost in dq.popleft()[1]:
                    est["pe"] += cost

        def drain_tails(n, force=False):
            _drain(tails, n, force)
            _drain(tails2, n, force)

        def flush_tails():
            # only tail1 (acc readers) must precede a new chunk's ctx; tail2
            # (out-proj) is exempt and drains lazily off the diag hot region
            _drain(tails, len(tails), True)

        # ---- QKV projection windows ----
        def q_unit(w, xw):
            p_ps = sm_ps.tile([P, WN], F32, name="p_ps", tag="sm")
            for kc in range(KC):
                nc.tensor.matmul(p_ps, wq_s[:, kc, :], xw[:, kc, :],
                                 start=(kc == 0), stop=(kc == KC - 1))
                yield WN * 0.42
            nc.vector.tensor_copy(qT[:, w * WN:(w + 1) * WN], p_ps)
            yield 0.0

        def k_unit(w, st, xw):
            t0 = w * WN + st * P
            p_ps = sm_ps.tile([P, P], F32, name="kp", tag="sm")
            for kc in range(KC):
                nc.tensor.matmul(p_ps, wk_s[:, kc, :],
                                 xw[:, kc, st * P:(st + 1) * P],
                                 start=(kc == 0), stop=(kc == KC - 1))
                if kc % 2 == 1:
                    yield 2 * P * 0.42
            nc.vector.tensor_copy(kT[:, t0:t0 + P], p_ps)
            yield 0.0

        def v_unit(w, st, xw):
            jt = (w * WN) // P + st
            vp = sm_ps.tile([P, CW], F32, name="vp", tag="sm")
            for kc in range(KC):
                nc.tensor.matmul(vp, xw[:, kc, st * P:(st + 1) * P],
                                 wv_s[:, kc, :],
                                 start=(kc == 0), stop=(kc == KC - 1))
                if kc % 2 == 1:
                    yield 2 * CW * 0.42
            # one strided copy writes both head halves around the
            # preserved ones-column at col 64
            nc.vector.tensor_copy(vA[:, jt, 0:130].rearrange(
                "p (two xx) -> p two xx", two=2)[:, :, 0:64],
                vp.rearrange("p (two xx) -> p two xx", two=2))
            yield 0.0

        state = {"enqueued": 0}
        uid_q = {}
        uid_k = {}
        uid_v = {}

        def add_unit(gen):
            uid = est["uid"]
            est["uid"] += 1
            work.append((uid, gen))
            return uid

        def enqueue_window(upto):
            while state["enqueued"] < min(upto, T // WN):
                w = state["enqueued"]
                xw = xw_pool.tile([P, KC, WN], BF16, name="xw", tag="xw")
                # 2 kc-chunks per dma_start: halves SP dispatch cost while
                # still spreading the window across parallel DMA engines
                for kc in range(0, KC, 2):
                    src = bass.AP(tensor=xT.tensor, offset=kc * P * T + w * WN,
                                  ap=[[T, P], [P * T, 2], [1, WN]])
                    nc.sync.dma_start(out=xw[:, kc:kc + 2, :], in_=src)
                if w == 0:
                    # deferred: not needed before the first attention chunk
                    nc.scalar.dma_start(out=wo_s, in_=wo)
                    nc.scalar.dma_start(out=mask_s, in_=mask)
                    nc.scalar.dma_start(out=ident_s, in_=ident)
                uid_q[w] = add_unit(q_unit(w, xw))
                for st in range(WN // P):
                    gt = (w * WN) // P + st
                    uid_k[gt] = add_unit(k_unit(w, st, xw))
                    uid_v[gt] = add_unit(v_unit(w, st, xw))
                state["enqueued"] += 1

        # ---- per-query-tile tail: normalize + transpose + out-proj ----
        def tail1_unit(b, icn, qs):
            acc = accA if qs < 2 else accB
            i2 = qs % 2
            gq0 = b * S + (icn * QT + qs) * P
            rcp = rcp_sb.tile([P, 2, 1], F32, name="rcp", tag="rcp")
            nc.vector.reciprocal(rcp, acc[:, i2, :, 64:65])
            nrmd = nrm_sb.tile([P, P], BF16, name="nrmd", tag="nrmd")
            for h in range(2):
                nc.vector.tensor_scalar_mul(
                    nrmd[:, h * 64:(h + 1) * 64],
                    acc[:, i2, h, 0:64], rcp[:, h, :])
            yield 0.0
            tT = sm_ps.tile([P, P], BF16, name="tT", tag="sm")
            nc.tensor.transpose(tT, nrmd, ident_s)
            nc.vector.tensor_copy(cT[:, gq0:gq0 + P], tT)
            yield P * 0.42

        def tail2_unit(b, icn, qs, split_evac=False):
            gq0 = b * S + (icn * QT + qs) * P
            ob = ob_sb.tile([P, D], BF16, name="ob", tag="ob")
            for nn in range(D // 512):
                op = sm_ps.tile([P, 512], F32, name="op", tag="sm")
                nc.tensor.matmul(op, cT[:, gq0:gq0 + P],
                                 wo_s[:, nn * 512:(nn + 1) * 512],
                                 start=True, stop=True)
                if split_evac and nn % 2 == 0:
                    # ACT is idle at the kernel tail: split the evacuation
                    nc.scalar.activation(ob[:, nn * 512:(nn + 1) * 512], op,
                                         AF.Copy)
                else:
                    nc.vector.tensor_copy(ob[:, nn * 512:(nn + 1) * 512], op)
                yield 512 * 0.42
            # out-stores go via the idle Pool engine's SWDGE queue: their
            # data-ready waits must not block the in-order SP/HWDGE queue
            # that feeds latency-critical xw loads. Exception: the last
            # chunk's stores use the SP queue (empty by then) — the final
            # drain barrier otherwise waits ~1us of SWDGE descriptor
            # generation per store, serial on Pool
            if split_evac:
                nc.sync.dma_start(out=out[gq0:gq0 + P, :], in_=ob)
            else:
                nc.gpsimd.dma_start(out=out[gq0:gq0 + P, :], in_=ob)

        # ---- attention chunks ----
        def emit_ctx(b, icn, jt, ex):
            d = jt - icn * QT
            need_until(uid_v[b * NJ + jt])
            if jt == 0:
                # new chunk touches every acc region: prior chunks' tail
                # reads must already be emitted (WAR via program order);
                # a little filler covers the DVE normalize latency so the
                # first ctx doesn't stall on the acc WAR
                flush_tails()
                for _ in range(BFILL):
                    if work:
                        pump_one()
            for h in range(2):
                for qs in range(max(d, 0), QT):
                    acc = accA if qs < 2 else accB
                    # one start=True per acc bank per chunk: start pends the
                    # whole 2KB zero region, so the other groups' first
                    # accumulate materializes zero+write (lazy per-byte).
                    # All writers are PE matmuls -> program order holds.
                    nc.tensor.matmul(
                        acc[:, qs % 2, h, :],
                        ex[:, h, qs * P:(qs + 1) * P],
                        vA[:, b * NJ + jt, h * 65:(h + 1) * 65],
                        start=(jt == 0 and h == 0 and qs % 2 == 0),
                        stop=(jt == icn * QT + qs),
                        skip_group_check=True)
                    est["pe"] += 65 * 0.42
            if d >= 0:
                last = (b == B - 1 and icn == NIC - 1)
                tails.append((est["iter"] + TAIL_LAG, tail1_unit(b, icn, d)))
                tails2.append((est["iter"] + T2_LAG,
                               tail2_unit(b, icn, d, split_evac=last)))

        # lag-2 software pipeline: ctx(jt-2) is emitted after scores(jt), so
        # PE never reaches a ctx matmul before its exp (ACT, ~1.07us after
        # its scores) has retired; pend carries across chunk boundaries
        # p-state warmup: the PE ramps to full clock only after 3us of
        # continuous execution; burn the initial xw-DMA wait on dummy
        # matmuls so the first projections run at full speed
        for i in range(int(os.environ.get('KV2_WARM', '4'))):
            wp = sm_ps.tile([P, 512], F32, name="wp", tag="sm")
            nc.tensor.matmul(wp, warm[:, 0:P], warm, start=True, stop=True)

        pend = deque()
        for b in range(B):
            for icn in range(NIC):
                # per-chunk meter reset: a cumulative surplus from the
                # PE-bound early phase must not block pre-draining filler
                # during ACT-bound chunks
                est["pe"] = est["act"] = 0.0
                gi0 = b * S + icn * IC
                njt = (icn + 1) * QT
                nw = (b * S + (icn + 1) * IC + WN - 1) // WN
                enqueue_window(nw + AHEAD)
                need_until(uid_q[(b * S + icn * IC) // WN])
                last_chunk = (b == B - 1 and icn == NIC - 1)
                for jt in range(njt):
                    est["iter"] += 1
                    if icn >= 5:
                        # ACT-bound chunk: filler first, so PE doesn't sit
                        # stalled on the sc-buffer WAR (exp jt-2) and reset
                        # its p-state ramp
                        pump_to_balance()
                    # scores+exp go out first each iteration so the ACT
                    # engine's exp queue never waits behind tail/filler work
                    need_until(uid_k[b * NJ + jt])
                    il0 = max(0, jt * P - icn * IC)
                    gj0 = b * S + jt * P
                    sc = sc_ps.tile([P, 2, IC], F32, name="sc", tag="sc")
                    for h in range(2):
                        hp = h * 64
                        nc.tensor.matmul(
                            sc[:, h, il0:IC],
                            kT[hp:hp + 64, gj0:gj0 + P],
                            qT[hp:hp + 64, gi0 + il0:gi0 + IC],
                            start=True, stop=True)
                    est["pe"] += 2 * (IC - il0) * 0.42
                    ex = ex_sb.tile([P, 2, IC], BF16, name="ex", tag="ex")
                    nc.scalar.activation(ex[:, :, il0:IC], sc[:, :, il0:IC],
                                         AF.Exp, scale=0.125)
                    est["act"] += 2 * (IC - il0) * 0.84 + 185
                    if jt * P >= icn * IC:  # diagonal tile
                        for h in range(2):
                            nc.vector.tensor_mul(ex[:, h, il0:il0 + P],
                                                 ex[:, h, il0:il0 + P],
                                                 mask_s)
                    if last_chunk:
                        # no later chunk will absorb the backlog: drain all
                        # ready tails now so the end tail stays short
                        drain_tails(4)
                        if jt >= njt - 8:
                            flush_tails()
                            _drain(tails2, len(tails2), True)
                    else:
                        drain_tails(NTAILS)
                    if jt == njt - 2 and not last_chunk:
                        # flush the next chunk's q-window now, while PE is
                        # hot (full p-state) and the diag region's ACT load
                        # is light — the same flush after the boundary stall
                        # would run at half clock
                        b2, icn2 = (b, icn + 1) if icn + 1 < NIC else (b + 1, 0)
                        need_until(uid_q[(b2 * S + icn2 * IC) // WN])
                        need_until(uid_k[b2 * NJ])
                        need_until(uid_v[b2 * NJ])
                    pend.append((b, icn, jt, ex))
                    # deeper ctx lag in short early chunks: gives the prev
                    # chunk's last tail1 normalize more slack before the
                    # first ctx of this chunk hits the acc WAR
                    plag = 3 if icn <= int(os.environ.get('KV2_PLAGC', '7')) else 2
                    if len(pend) > plag:
                        emit_ctx(*pend.popleft())
                    pump_to_balance()
                # drain the ctx pipeline at chunk end so the last tiles'
                # tail1 normalizes get a couple of iterations of slack
                # before the next chunk's first ctx (acc WAR); filler first
                # so the last ctx doesn't stall on its just-emitted exp
                for _ in range(BFILL):
                    if work:
                        pump_one()
                while pend:
                    emit_ctx(*pend.popleft())
        flush_tails()
        _drain(tails2, len(tails2), True)
        while work:
            pump_one()

    nc.compile()
    return nc


def _warrange(w, bf16):
    # [D, CW] -> [P, D//P, CW] contiguous (the SBUF layout, so the DMA is
    # a single contiguous copy instead of 256B strided pieces)
    D, CW_ = w.shape
    return np.ascontiguousarray(
        w.reshape(D // P, P, CW_).transpose(1, 0, 2)).astype(bf16)


def make_in_maps(x, Wq, Wk, Wv, Wo):
    import ml_dtypes
    bf16 = ml_dtypes.bfloat16
    B, S, D = x.shape
    xT = np.ascontiguousarray(x.reshape(B * S, D).T).astype(bf16)
    mask = np.triu(np.ones((P, P), dtype=bf16))
    ident = np.eye(P, dtype=bf16)
    in_maps = []
    for c in range(N_CORES):
        cs = slice(c * CW, (c + 1) * CW)
        in_maps.append({
            "xT": xT,
            "wq": _warrange(Wq[:, cs], bf16),
            "wk": _warrange(Wk[:, cs], bf16),
            "wv": _warrange(Wv[:, cs], bf16),
            "wo": np.ascontiguousarray(Wo[cs, :]).astype(bf16),
            "mask": mask,
            "ident": ident,
        })
    return in_maps


_CACHED_NC = None


def kernel(x, Wq, Wk, Wv, Wo, bo, _trace=False):
    global _CACHED_NC
    x = np.asarray(x, dtype=np.float32)
    B, S, D = x.shape
    if _CACHED_NC is None:
        _CACHED_NC = build_program(S=S, B=B, D=D)
    nc = _CACHED_NC
    in_maps = make_in_maps(x, np.asarray(Wq), np.asarray(Wk),
                           np.asarray(Wv), np.asarray(Wo))
    res = None
    for attempt in range(3):
        try:
            res = run_bass_kernel_spmd(nc, in_maps, list(range(N_CORES)),
                                       trace=_trace)
            break
        except Exception:
            if attempt == 2:
                raise
    out = np.zeros((B * S, D), dtype=np.float32)
    for c in range(N_CORES):
        out += res.results[c]["out"].astype(np.float32)
    out += np.asarray(bo, dtype=np.float32)[None, :]
    if _trace:
        kernel._last_result = res
    return out.reshape(B, S, D)


# revision 8
# speedup vs baseline: 1.3101x; 1.0022x over previous
"""Multi-head causal attention (B=2, S=4096, D=1024, H=16) on 8 TRN2 NeuronCores.

Sharding: head-parallel. Core c computes heads 2c, 2c+1 (128 of the 1024
projection columns) for both batches:
  - QKV column-parallel: each core gets Wq/Wk/Wv[:, c*128:(c+1)*128]
  - out-proj row-parallel: partial_out = ctx_c @ Wo[c*128:(c+1)*128, :]
  - host sums the 8 partials and adds bo.

The TimelineSim cost model charges a matmul only by its OUTPUT free size
(N columns), so the ctx product is blocked transposed: per (128-query x
128-key) causal block, out[128 q, 65] += ex[128 k, 128 q].T @ vA[128 k, 65]
streams just 65 columns (the 65th is an all-ones column producing the
softmax denominator), instead of re-streaming 512 query columns per key
tile. The denominator then sits on the free dim, so normalization is a
cheap per-partition tensor_scalar, and a single PE transpose per query
tile restores the [cdim, token] layout for the out-projection.

Layouts on-chip (per core):
  qT, kT:  [128, T]  rows 0:64 head0, 64:128 head1 (transposed projections)
  vA:      [128, T/128, 130]  per key-tile [v_h0 | ones | v_h1 | ones]
  sc/ex:   [128 keys, 2 heads, 512 queries]  PSUM scores / SBUF exp(bf16)
  acc:     PSUM [128 q, 2 qsub, 2 heads, 65] x2 (unnormalized ctx + denom)
  cT:      [128 cdim, T]  normalized ctx, transposed back per 128-q tile

Emission keeps both bottleneck engines (PE ~283us of matmul columns,
ACT ~280us of exp) near-continuously busy: QKV projection windows are
generators on a work queue, metered one matmul at a time between
attention iterations (forced just-in-time by fine-grained need-barriers,
pre-drained during ACT-bound chunks); per-tile tail work drains with an
iteration lag so its cross-engine latency chains stay off the PE stream;
out-stores ride the idle Pool engine's SWDGE queue so their data waits
never block the in-order SP queue feeding xw loads; deep SBUF pools
(ex/nrm/rcp/ob) keep WAR rotations off the critical path; dummy warm-up
matmuls ramp the PE p-state during the initial DMA fill.
"""

import os
from collections import deque
from contextlib import ExitStack

import numpy as np

import concourse.bass as bass
import concourse.tile as tile
from concourse import bacc, mybir
from concourse.bass_utils import run_bass_kernel_spmd

F32 = mybir.dt.float32
BF16 = mybir.dt.bfloat16
P = 128
AF = mybir.ActivationFunctionType

N_CORES = 8
B_FULL, S_FULL, D_FULL, H_FULL = 2, 4096, 1024, 16
DH = 64
CW = 128  # projection columns per core (2 heads * 64)


def build_program(S=S_FULL, B=B_FULL, D=D_FULL):
    """Build the per-core Bass program (same program on all 8 cores)."""
    T = B * S
    KC = D // P            # contraction chunks for the projections
    IC = min(512, S)       # query-chunk width
    QT = IC // P           # query tiles per chunk (4)
    NJ = S // P            # key tiles per batch
    NIC = S // IC          # query chunks per batch
    WN = min(512, T)       # QKV token window

    nc = bacc.Bacc("TRN2", target_bir_lowering=False, debug=False,
                   num_devices=N_CORES)

    xT = nc.dram_tensor("xT", [D, T], BF16, kind="ExternalInput").ap()
    wq = nc.dram_tensor("wq", [P, D // P, CW], BF16, kind="ExternalInput").ap()
    wk = nc.dram_tensor("wk", [P, D // P, CW], BF16, kind="ExternalInput").ap()
    wv = nc.dram_tensor("wv", [P, D // P, CW], BF16, kind="ExternalInput").ap()
    wo = nc.dram_tensor("wo", [CW, D], BF16, kind="ExternalInput").ap()
    mask = nc.dram_tensor("mask", [P, P], BF16, kind="ExternalInput").ap()
    ident = nc.dram_tensor("ident", [P, P], BF16, kind="ExternalInput").ap()
    out = nc.dram_tensor("out", [T, D], BF16, kind="ExternalOutput").ap()

    with tile.TileContext(nc) as tc, ExitStack() as ctx:
        singles = ctx.enter_context(tc.tile_pool(name="singles", bufs=1))
        qT = singles.tile([P, T], BF16, name="qT")
        kT = singles.tile([P, T], BF16, name="kT")
        vA = singles.tile([P, B * NJ, 130], BF16, name="vA")
        cT = singles.tile([P, T], BF16, name="cT")
        wq_s = singles.tile([P, KC, CW], BF16, name="wq_s")
        wk_s = singles.tile([P, KC, CW], BF16, name="wk_s")
        wv_s = singles.tile([P, KC, CW], BF16, name="wv_s")
        wo_s = singles.tile([CW, D], BF16, name="wo_s")
        mask_s = singles.tile([P, P], BF16, name="mask_s")
        ident_s = singles.tile([P, P], BF16, name="ident_s")

        # weight loads dispatch on the ACT hwdge queue, in parallel with
        # the SP queue's first xw window loads
        nc.scalar.dma_start(out=wq_s, in_=wq)
        nc.scalar.dma_start(out=wk_s, in_=wk)
        nc.scalar.dma_start(out=wv_s, in_=wv)
        nc.vector.memset(vA[:, :, 64:65], 1.0)
        nc.vector.memset(vA[:, :, 129:130], 1.0)
        warm = singles.tile([P, 512], BF16, name="warm")
        nc.vector.memset(warm, 0.0)

        # PSUM budget (8 banks): sc 2x2 + acc 2x1 + shared-small 2x1 = 8
        ps_single = ctx.enter_context(
            tc.tile_pool(name="ps_single", bufs=1, space=bass.MemorySpace.PSUM))
        # [q, qsub-pair, head, v+denom]; accA holds qsub 0,1; accB 2,3
        accA = ps_single.tile([P, 2, 2, 65], F32, name="accA")
        accB = ps_single.tile([P, 2, 2, 65], F32, name="accB")
        sc_ps = ctx.enter_context(
            tc.tile_pool(name="sc_ps", bufs=2, space=bass.MemorySpace.PSUM))
        sm_ps = ctx.enter_context(
            tc.tile_pool(name="sm_ps", bufs=2, space=bass.MemorySpace.PSUM))
        xw_pool = ctx.enter_context(tc.tile_pool(name="xw_pool", bufs=10))
        ex_sb = ctx.enter_context(tc.tile_pool(name="ex_sb", bufs=8))
        nrm_sb = ctx.enter_context(tc.tile_pool(name="nrm_sb", bufs=8))
        rcp_sb = ctx.enter_context(tc.tile_pool(name="rcp_sb", bufs=8))
        ob_sb = ctx.enter_context(tc.tile_pool(name="ob_sb", bufs=4))

        # ---- deferred-work queue of generators yielding ~PE-ns steps ----
        # Filler (QKV windows) is metered one matmul at a time between
        # attention iterations so the ACT engine's exp queue never runs dry
        # and PE never stalls (a PE stall resets the p-state ramp to half
        # clock for 3us). Tail units (normalize/transpose/out-proj) are
        # latency chains: they drain with priority, max 2 per iteration.
        import os
        work = deque()   # entries: (uid, generator)
        tails = deque()  # entries: (ready_iter, generator)
        tails2 = deque()  # out-proj units: lazy, never forced at boundaries
        est = {"pe": 0.0, "act": 0.0, "uid": 0, "done": -1, "iter": 0}
        MARGIN = float(os.environ.get("KV2_MARGIN", "0"))
        TAIL_LAG = int(os.environ.get("KV2_TAIL_LAG", "4"))
        T2_LAG = int(os.environ.get("KV2_T2_LAG", "5"))
        AHEAD = int(os.environ.get("KV2_AHEAD", "4"))
        NTAILS = int(os.environ.get("KV2_NTAILS", "2"))
        BFILL = int(os.environ.get("KV2_BFILL", "0"))

        def pump_one():
            uid, gen = work[0]
            try:
                est["pe"] += next(gen)
            except StopIteration:
                work.popleft()
                est["done"] = uid

        def pump_to_balance():
            while work and est["pe"] < est["act"] + MARGIN:
                pump_one()
            if not work and est["pe"] < est["act"]:
                # queue ran dry: the PE deficit is past idle time, not
                # fillable later — don't let it trigger a future dump
                est["pe"] = est["act"]

        def need_until(uid):
            while est["done"] < uid:
                pump_one()

        def _drain(dq, n, force):
            for _ in range(min(n, len(dq))):
                if not force and dq[0][0] > est["iter"]:
                    return
                for cost in dq.popleft()[1]:
                    est["pe"] += cost

        def drain_tails(n, force=False):
            _drain(tails, n, force)
            _drain(tails2, n, force)

        def flush_tails():
            # only tail1 (acc readers) must precede a new chunk's ctx; tail2
            # (out-proj) is exempt and drains lazily off the diag hot region
            _drain(tails, len(tails), True)

        # ---- QKV projection windows ----
        def q_unit(w, xw):
            p_ps = sm_ps.tile([P, WN], F32, name="p_ps", tag="sm")
            for kc in range(KC):
                nc.tensor.matmul(p_ps, wq_s[:, kc, :], xw[:, kc, :],
                                 start=(kc == 0), stop=(kc == KC - 1))
                yield WN * 0.42
            nc.vector.tensor_copy(qT[:, w * WN:(w + 1) * WN], p_ps)
            yield 0.0

        def k_unit(w, st, xw):
            t0 = w * WN + st * P
            p_ps = sm_ps.tile([P, P], F32, name="kp", tag="sm")
            for kc in range(KC):
                nc.tensor.matmul(p_ps, wk_s[:, kc, :],
                                 xw[:, kc, st * P:(st + 1) * P],
                                 start=(kc == 0), stop=(kc == KC - 1))
                if kc % 2 == 1:
                    yield 2 * P * 0.42
            nc.vector.tensor_copy(kT[:, t0:t0 + P], p_ps)
            yield 0.0

        def v_unit(w, st, xw):
            jt = (w * WN) // P + st
            vp = sm_ps.tile([P, CW], F32, name="vp", tag="sm")
            for kc in range(KC):
                nc.tensor.matmul(vp, xw[:, kc, st * P:(st + 1) * P],
                                 wv_s[:, kc, :],
                                 start=(kc == 0), stop=(kc == KC - 1))
                if kc % 2 == 1:
                    yield 2 * CW * 0.42
            # one strided copy writes both head halves around the
            # preserved ones-column at col 64
            nc.vector.tensor_copy(vA[:, jt, 0:130].rearrange(
                "p (two xx) -> p two xx", two=2)[:, :, 0:64],
                vp.rearrange("p (two xx) -> p two xx", two=2))
            yield 0.0

        state = {"enqueued": 0}
        uid_q = {}
        uid_k = {}
        uid_v = {}

        def add_unit(gen):
            uid = est["uid"]
            est["uid"] += 1
            work.append((uid, gen))
            return uid

        def enqueue_window(upto):
            while state["enqueued"] < min(upto, T // WN):
                w = state["enqueued"]
                xw = xw_pool.tile([P, KC, WN], BF16, name="xw", tag="xw")
                # 2 kc-chunks per dma_start: halves SP dispatch cost while
                # still spreading the window across parallel DMA engines
                for kc in range(0, KC, 2):
                    src = bass.AP(tensor=xT.tensor, offset=kc * P * T + w * WN,
                                  ap=[[T, P], [P * T, 2], [1, WN]])
                    nc.sync.dma_start(out=xw[:, kc:kc + 2, :], in_=src)
                if w == 0:
                    # deferred: not needed before the first attention chunk
                    nc.scalar.dma_start(out=wo_s, in_=wo)
                    nc.scalar.dma_start(out=mask_s, in_=mask)
                    nc.scalar.dma_start(out=ident_s, in_=ident)
                uid_q[w] = add_unit(q_unit(w, xw))
                for st in range(WN // P):
                    gt = (w * WN) // P + st
                    uid_k[gt] = add_unit(k_unit(w, st, xw))
                    uid_v[gt] = add_unit(v_unit(w, st, xw))
                state["enqueued"] += 1

        # ---- per-query-tile tail: normalize + transpose + out-proj ----
        def tail1_unit(b, icn, qs):
            acc = accA if qs < 2 else accB
            i2 = qs % 2
            gq0 = b * S + (icn * QT + qs) * P
            rcp = rcp_sb.tile([P, 2, 1], F32, name="rcp", tag="rcp")
            nc.vector.reciprocal(rcp, acc[:, i2, :, 64:65])
            nrmd = nrm_sb.tile([P, P], BF16, name="nrmd", tag="nrmd")
            for h in range(2):
                nc.vector.tensor_scalar_mul(
                    nrmd[:, h * 64:(h + 1) * 64],
                    acc[:, i2, h, 0:64], rcp[:, h, :])
            yield 0.0
            tT = sm_ps.tile([P, P], BF16, name="tT", tag="sm")
            nc.tensor.transpose(tT, nrmd, ident_s)
            nc.vector.tensor_copy(cT[:, gq0:gq0 + P], tT)
            yield P * 0.42

        def tail2_unit(b, icn, qs, split_evac=False):
            gq0 = b * S + (icn * QT + qs) * P
            ob = ob_sb.tile([P, D], BF16, name="ob", tag="ob")
            for nn in range(D // 512):
                op = sm_ps.tile([P, 512], F32, name="op", tag="sm")
                nc.tensor.matmul(op, cT[:, gq0:gq0 + P],
                                 wo_s[:, nn * 512:(nn + 1) * 512],
                                 start=True, stop=True)
                if split_evac and nn % 2 == 0:
                    # ACT is idle at the kernel tail: split the evacuation
                    nc.scalar.activation(ob[:, nn * 512:(nn + 1) * 512], op,
                                         AF.Copy)
                else:
                    nc.vector.tensor_copy(ob[:, nn * 512:(nn + 1) * 512], op)
                yield 512 * 0.42
            # out-stores go via the idle Pool engine's SWDGE queue: their
            # data-ready waits must not block the in-order SP/HWDGE queue
            # that feeds latency-critical xw loads. Exception: the last
            # chunk's stores use the SP queue (empty by then) — the final
            # drain barrier otherwise waits ~1us of SWDGE descriptor
            # generation per store, serial on Pool
            if split_evac:
                nc.sync.dma_start(out=out[gq0:gq0 + P, :], in_=ob)
            else:
                nc.gpsimd.dma_start(out=out[gq0:gq0 + P, :], in_=ob)

        # ---- attention chunks ----
        def emit_ctx(b, icn, jt, ex):
            d = jt - icn * QT
            need_until(uid_v[b * NJ + jt])
            if jt == 0:
                # new chunk touches every acc region: prior chunks' tail
                # reads must already be emitted (WAR via program order);
                # a little filler covers the DVE normalize latency so the
                # first ctx doesn't stall on the acc WAR
                flush_tails()
                for _ in range(BFILL):
                    if work:
                        pump_one()
            for h in range(2):
                for qs in range(max(d, 0), QT):
                    acc = accA if qs < 2 else accB
                    # one start=True per acc bank per chunk: start pends the
                    # whole 2KB zero region, so the other groups' first
                    # accumulate materializes zero+write (lazy per-byte).
                    # All writers are PE matmuls -> program order holds.
                    nc.tensor.matmul(
                        acc[:, qs % 2, h, :],
                        ex[:, h, qs * P:(qs + 1) * P],
                        vA[:, b * NJ + jt, h * 65:(h + 1) * 65],
                        start=(jt == 0 and h == 0 and qs % 2 == 0),
                        stop=(jt == icn * QT + qs),
                        skip_group_check=True)
                    est["pe"] += 65 * 0.42
            if d >= 0:
                last = (b == B - 1 and icn == NIC - 1)
                tails.append((est["iter"] + TAIL_LAG, tail1_unit(b, icn, d)))
                tails2.append((est["iter"] + T2_LAG,
                               tail2_unit(b, icn, d, split_evac=last)))

        # lag-2 software pipeline: ctx(jt-2) is emitted after scores(jt), so
        # PE never reaches a ctx matmul before its exp (ACT, ~1.07us after
        # its scores) has retired; pend carries across chunk boundaries
        # p-state warmup: the PE ramps to full clock only after 3us of
        # continuous execution; burn the initial xw-DMA wait on dummy
        # matmuls so the first projections run at full speed
        for i in range(int(os.environ.get('KV2_WARM', '4'))):
            wp = sm_ps.tile([P, 512], F32, name="wp", tag="sm")
            nc.tensor.matmul(wp, warm[:, 0:P], warm, start=True, stop=True)

        pend = deque()
        for b in range(B):
            for icn in range(NIC):
                # per-chunk meter reset: a cumulative surplus from the
                # PE-bound early phase must not block pre-draining filler
                # during ACT-bound chunks
                est["pe"] = est["act"] = 0.0
                gi0 = b * S + icn * IC
                njt = (icn + 1) * QT
                nw = (b * S + (icn + 1) * IC + WN - 1) // WN
                enqueue_window(nw + AHEAD)
                need_until(uid_q[(b * S + icn * IC) // WN])
                last_chunk = (b == B - 1 and icn == NIC - 1)
                for jt in range(njt):
                    est["iter"] += 1
                    if icn >= 5:
                        # ACT-bound chunk: filler first, so PE doesn't sit
                        # stalled on the sc-buffer WAR (exp jt-2) and reset
                        # its p-state ramp
                        pump_to_balance()
                    # scores+exp go out first each iteration so the ACT
                    # engine's exp queue never waits behind tail/filler work
                    need_until(uid_k[b * NJ + jt])
                    il0 = max(0, jt * P - icn * IC)
                    gj0 = b * S + jt * P
                    sc = sc_ps.tile([P, 2, IC], F32, name="sc", tag="sc")
                    for h in range(2):
                        hp = h * 64
                        nc.tensor.matmul(
                            sc[:, h, il0:IC],
                            kT[hp:hp + 64, gj0:gj0 + P],
                            qT[hp:hp + 64, gi0 + il0:gi0 + IC],
                            start=True, stop=True)
                    est["pe"] += 2 * (IC - il0) * 0.42
                    ex = ex_sb.tile([P, 2, IC], BF16, name="ex", tag="ex")
                    nc.scalar.activation(ex[:, :, il0:IC], sc[:, :, il0:IC],
                                         AF.Exp, scale=0.125)
                    est["act"] += 2 * (IC - il0) * 0.84 + 185
                    if jt * P >= icn * IC:  # diagonal tile
                        # one strided op masks both heads: the mask operand
                        # repeats via a stride-0 middle dim
                        mb2 = bass.AP(tensor=mask_s.tensor,
                                      offset=mask_s.offset,
                                      ap=[mask_s.ap[0], [0, 2],
                                          mask_s.ap[-1]])
                        nc.vector.tensor_mul(ex[:, :, il0:il0 + P],
                                             ex[:, :, il0:il0 + P], mb2)
                    if last_chunk:
                        # no later chunk will absorb the backlog: drain all
                        # ready tails now so the end tail stays short
                        drain_tails(4)
                        if jt >= njt - 8:
                            flush_tails()
                            _drain(tails2, len(tails2), True)
                    else:
                        drain_tails(NTAILS)
                    if jt == njt - 2 and not last_chunk:
                        # flush the next chunk's q-window now, while PE is
                        # hot (full p-state) and the diag region's ACT load
                        # is light — the same flush after the boundary stall
                        # would run at half clock
                        b2, icn2 = (b, icn + 1) if icn + 1 < NIC else (b + 1, 0)
                        need_until(uid_q[(b2 * S + icn2 * IC) // WN])
                        need_until(uid_k[b2 * NJ])
                        need_until(uid_v[b2 * NJ])
                    pend.append((b, icn, jt, ex))
                    # deeper ctx lag in short early chunks: gives the prev
                    # chunk's last tail1 normalize more slack before the
                    # first ctx of this chunk hits the acc WAR
                    plag = 3 if icn <= int(os.environ.get('KV2_PLAGC', '7')) else 2
                    if len(pend) > plag:
                        emit_ctx(*pend.popleft())
                    pump_to_balance()
                # drain the ctx pipeline at chunk end so the last tiles'
                # tail1 normalizes get a couple of iterations of slack
                # before the next chunk's first ctx (acc WAR); filler first
                # so the last ctx doesn't stall on its just-emitted exp
                for _ in range(BFILL):
                    if work:
                        pump_one()
                while pend:
                    emit_ctx(*pend.popleft())
        flush_tails()
        _drain(tails2, len(tails2), True)
        while work:
            pump_one()

    nc.compile()
    return nc


def _warrange(w, bf16):
    # [D, CW] -> [P, D//P, CW] contiguous (the SBUF layout, so the DMA is
    # a single contiguous copy instead of 256B strided pieces)
    D, CW_ = w.shape
    return np.ascontiguousarray(
        w.reshape(D // P, P, CW_).transpose(1, 0, 2)).astype(bf16)


def make_in_maps(x, Wq, Wk, Wv, Wo):
    import ml_dtypes
    bf16 = ml_dtypes.bfloat16
    B, S, D = x.shape
    xT = np.ascontiguousarray(x.reshape(B * S, D).T).astype(bf16)
    mask = np.triu(np.ones((P, P), dtype=bf16))
    ident = np.eye(P, dtype=bf16)
    in_maps = []
    for c in range(N_CORES):
        cs = slice(c * CW, (c + 1) * CW)
        in_maps.append({
            "xT": xT,
            "wq": _warrange(Wq[:, cs], bf16),
            "wk": _warrange(Wk[:, cs], bf16),
            "wv": _warrange(Wv[:, cs], bf16),
            "wo": np.ascontiguousarray(Wo[cs, :]).astype(bf16),
            "mask": mask,
            "ident": ident,
        })
    return in_maps


_CACHED_NC = None


def kernel(x, Wq, Wk, Wv, Wo, bo, _trace=False):
    global _CACHED_NC
    x = np.asarray(x, dtype=np.float32)
    B, S, D = x.shape
    if _CACHED_NC is None:
        _CACHED_NC = build_program(S=S, B=B, D=D)
    nc = _CACHED_NC
    in_maps = make_in_maps(x, np.asarray(Wq), np.asarray(Wk),
                           np.asarray(Wv), np.asarray(Wo))
    res = None
    for attempt in range(3):
        try:
            res = run_bass_kernel_spmd(nc, in_maps, list(range(N_CORES)),
                                       trace=_trace)
            break
        except Exception:
            if attempt == 2:
                raise
    out = np.zeros((B * S, D), dtype=np.float32)
    for c in range(N_CORES):
        out += res.results[c]["out"].astype(np.float32)
    out += np.asarray(bo, dtype=np.float32)[None, :]
    if _trace:
        kernel._last_result = res
    return out.reshape(B, S, D)


# revision 9
# speedup vs baseline: 1.3273x; 1.0131x over previous
"""Multi-head causal attention (B=2, S=4096, D=1024, H=16) on 8 TRN2 NeuronCores.

Sharding: head-parallel. Core c computes heads 2c, 2c+1 (128 of the 1024
projection columns) for both batches:
  - QKV column-parallel: each core gets Wq/Wk/Wv[:, c*128:(c+1)*128]
  - out-proj row-parallel: partial_out = ctx_c @ Wo[c*128:(c+1)*128, :]
  - host sums the 8 partials and adds bo.

The TimelineSim cost model charges a matmul only by its OUTPUT free size
(N columns), so the ctx product is blocked transposed: per (128-query x
128-key) causal block, out[128 q, 65] += ex[128 k, 128 q].T @ vA[128 k, 65]
streams just 65 columns (the 65th is an all-ones column producing the
softmax denominator), instead of re-streaming 512 query columns per key
tile. The denominator then sits on the free dim, so normalization is a
cheap per-partition tensor_scalar, and a single PE transpose per query
tile restores the [cdim, token] layout for the out-projection.

Layouts on-chip (per core):
  qT, kT:  [128, T]  rows 0:64 head0, 64:128 head1 (transposed projections)
  vA:      [128, T/128, 130]  per key-tile [v_h0 | ones | v_h1 | ones]
  sc/ex:   [128 keys, 2 heads, 512 queries]  PSUM scores / SBUF exp(bf16)
  acc:     PSUM [128 q, 2 qsub, 2 heads, 65] x2 (unnormalized ctx + denom)
  cT:      [128 cdim, T]  normalized ctx, transposed back per 128-q tile

Emission keeps both bottleneck engines (PE ~283us of matmul columns,
ACT ~280us of exp) near-continuously busy: QKV projection windows are
generators on a work queue, metered one matmul at a time between
attention iterations (forced just-in-time by fine-grained need-barriers,
pre-drained during ACT-bound chunks); per-tile tail work drains with an
iteration lag so its cross-engine latency chains stay off the PE stream;
out-stores ride the idle Pool engine's SWDGE queue so their data waits
never block the in-order SP queue feeding xw loads; deep SBUF pools
(ex/nrm/rcp/ob) keep WAR rotations off the critical path; dummy warm-up
matmuls ramp the PE p-state during the initial DMA fill.
"""

import os
from collections import deque
from contextlib import ExitStack

import numpy as np

import concourse.bass as bass
import concourse.tile as tile
from concourse import bacc, mybir
from concourse.bass_utils import run_bass_kernel_spmd

F32 = mybir.dt.float32
BF16 = mybir.dt.bfloat16
P = 128
AF = mybir.ActivationFunctionType

N_CORES = 8
B_FULL, S_FULL, D_FULL, H_FULL = 2, 4096, 1024, 16
DH = 64
CW = 128  # projection columns per core (2 heads * 64)


def build_program(S=S_FULL, B=B_FULL, D=D_FULL):
    """Build the per-core Bass program (same program on all 8 cores)."""
    T = B * S
    KC = D // P            # contraction chunks for the projections
    IC = min(512, S)       # query-chunk width
    QT = IC // P           # query tiles per chunk (4)
    NJ = S // P            # key tiles per batch
    NIC = S // IC          # query chunks per batch
    WN = min(512, T)       # QKV token window

    nc = bacc.Bacc("TRN2", target_bir_lowering=False, debug=False,
                   num_devices=N_CORES)

    xT = nc.dram_tensor("xT", [D, T], BF16, kind="ExternalInput").ap()
    wq = nc.dram_tensor("wq", [P, D // P, CW], BF16, kind="ExternalInput").ap()
    wk = nc.dram_tensor("wk", [P, D // P, CW], BF16, kind="ExternalInput").ap()
    wv = nc.dram_tensor("wv", [P, D // P, CW], BF16, kind="ExternalInput").ap()
    wo = nc.dram_tensor("wo", [CW, D], BF16, kind="ExternalInput").ap()
    mask = nc.dram_tensor("mask", [P, P], BF16, kind="ExternalInput").ap()
    ident = nc.dram_tensor("ident", [P, P], BF16, kind="ExternalInput").ap()
    out = nc.dram_tensor("out", [T, D], BF16, kind="ExternalOutput").ap()

    with tile.TileContext(nc) as tc, ExitStack() as ctx:
        singles = ctx.enter_context(tc.tile_pool(name="singles", bufs=1))
        qT = singles.tile([P, T], BF16, name="qT")
        kT = singles.tile([P, T], BF16, name="kT")
        vA = singles.tile([P, B * NJ, 130], BF16, name="vA")
        cT = singles.tile([P, T], BF16, name="cT")
        wq_s = singles.tile([P, KC, CW], BF16, name="wq_s")
        wk_s = singles.tile([P, KC, CW], BF16, name="wk_s")
        wv_s = singles.tile([P, KC, CW], BF16, name="wv_s")
        wo_s = singles.tile([CW, D], BF16, name="wo_s")
        mask_s = singles.tile([P, P], BF16, name="mask_s")
        ident_s = singles.tile([P, P], BF16, name="ident_s")

        # weight loads dispatch on the ACT hwdge queue, in parallel with
        # the SP queue's first xw window loads
        nc.scalar.dma_start(out=wq_s, in_=wq)
        nc.scalar.dma_start(out=wk_s, in_=wk)
        nc.scalar.dma_start(out=wv_s, in_=wv)
        nc.vector.memset(vA[:, :, 64:65], 1.0)
        nc.vector.memset(vA[:, :, 129:130], 1.0)
        warm = singles.tile([P, 512], BF16, name="warm")
        nc.vector.memset(warm, 0.0)

        # PSUM budget (8 banks): sc 2x2 + acc 2x1 + shared-small 2x1 = 8
        ps_single = ctx.enter_context(
            tc.tile_pool(name="ps_single", bufs=1, space=bass.MemorySpace.PSUM))
        # [q, qsub-pair, head, v+denom]; accA holds qsub 0,1; accB 2,3
        accA = ps_single.tile([P, 2, 2, 65], F32, name="accA")
        accB = ps_single.tile([P, 2, 2, 65], F32, name="accB")
        sc_ps = ctx.enter_context(
            tc.tile_pool(name="sc_ps", bufs=2, space=bass.MemorySpace.PSUM))
        sm_ps = ctx.enter_context(
            tc.tile_pool(name="sm_ps", bufs=2, space=bass.MemorySpace.PSUM))
        xw_pool = ctx.enter_context(tc.tile_pool(name="xw_pool", bufs=10))
        ex_sb = ctx.enter_context(tc.tile_pool(name="ex_sb", bufs=8))
        nrm_sb = ctx.enter_context(tc.tile_pool(name="nrm_sb", bufs=8))
        rcp_sb = ctx.enter_context(tc.tile_pool(name="rcp_sb", bufs=8))
        ob_sb = ctx.enter_context(tc.tile_pool(name="ob_sb", bufs=4))

        # ---- deferred-work queue of generators yielding ~PE-ns steps ----
        # Filler (QKV windows) is metered one matmul at a time between
        # attention iterations so the ACT engine's exp queue never runs dry
        # and PE never stalls (a PE stall resets the p-state ramp to half
        # clock for 3us). Tail units (normalize/transpose/out-proj) are
        # latency chains: they drain with priority, max 2 per iteration.
        import os
        work = deque()   # entries: (uid, generator)
        tails = deque()  # entries: (ready_iter, generator)
        tails2 = deque()  # out-proj units: lazy, never forced at boundaries
        est = {"pe": 0.0, "act": 0.0, "uid": 0, "done": -1, "iter": 0}
        MARGIN = float(os.environ.get("KV2_MARGIN", "0"))
        TAIL_LAG = int(os.environ.get("KV2_TAIL_LAG", "4"))
        T2_LAG = int(os.environ.get("KV2_T2_LAG", "5"))
        AHEAD = int(os.environ.get("KV2_AHEAD", "4"))
        NTAILS = int(os.environ.get("KV2_NTAILS", "2"))
        BFILL = int(os.environ.get("KV2_BFILL", "0"))

        def pump_one():
            uid, gen = work[0]
            try:
                est["pe"] += next(gen)
            except StopIteration:
                work.popleft()
                est["done"] = uid

        def pump_to_balance():
            while work and est["pe"] < est["act"] + MARGIN:
                pump_one()
            if not work and est["pe"] < est["act"]:
                # queue ran dry: the PE deficit is past idle time, not
                # fillable later — don't let it trigger a future dump
                est["pe"] = est["act"]

        def need_until(uid):
            while est["done"] < uid:
                pump_one()

        def _drain(dq, n, force):
            for _ in range(min(n, len(dq))):
                if not force and dq[0][0] > est["iter"]:
                    return
                for cost in dq.popleft()[1]:
                    est["pe"] += cost

        def drain_tails(n, force=False):
            _drain(tails, n, force)
            _drain(tails2, n, force)

        def flush_tails():
            # only tail1 (acc readers) must precede a new chunk's ctx; tail2
            # (out-proj) is exempt and drains lazily off the diag hot region
            _drain(tails, len(tails), True)

        # ---- QKV projection windows ----
        def q_unit(w, xw):
            p_ps = sm_ps.tile([P, WN], F32, name="p_ps", tag="sm")
            for kc in range(KC):
                nc.tensor.matmul(p_ps, wq_s[:, kc, :], xw[:, kc, :],
                                 start=(kc == 0), stop=(kc == KC - 1))
                yield WN * 0.42
            nc.vector.tensor_copy(qT[:, w * WN:(w + 1) * WN], p_ps)
            yield 0.0

        def k_unit(w, st, xw):
            t0 = w * WN + st * P
            p_ps = sm_ps.tile([P, P], F32, name="kp", tag="sm")
            for kc in range(KC):
                nc.tensor.matmul(p_ps, wk_s[:, kc, :],
                                 xw[:, kc, st * P:(st + 1) * P],
                                 start=(kc == 0), stop=(kc == KC - 1))
                if kc % 2 == 1:
                    yield 2 * P * 0.42
            nc.vector.tensor_copy(kT[:, t0:t0 + P], p_ps)
            yield 0.0

        def v_unit(w, st, xw):
            jt = (w * WN) // P + st
            vp = sm_ps.tile([P, CW], F32, name="vp", tag="sm")
            for kc in range(KC):
                nc.tensor.matmul(vp, xw[:, kc, st * P:(st + 1) * P],
                                 wv_s[:, kc, :],
                                 start=(kc == 0), stop=(kc == KC - 1))
                if kc % 2 == 1:
                    yield 2 * CW * 0.42
            # one strided copy writes both head halves around the
            # preserved ones-column at col 64
            nc.vector.tensor_copy(vA[:, jt, 0:130].rearrange(
                "p (two xx) -> p two xx", two=2)[:, :, 0:64],
                vp.rearrange("p (two xx) -> p two xx", two=2))
            yield 0.0

        state = {"enqueued": 0}
        uid_q = {}
        uid_k = {}
        uid_v = {}

        def add_unit(gen):
            uid = est["uid"]
            est["uid"] += 1
            work.append((uid, gen))
            return uid

        def enqueue_window(upto):
            while state["enqueued"] < min(upto, T // WN):
                w = state["enqueued"]
                xw = xw_pool.tile([P, KC, WN], BF16, name="xw", tag="xw")
                # 2 kc-chunks per dma_start: halves SP dispatch cost while
                # still spreading the window across parallel DMA engines
                for kc in range(0, KC, 2):
                    src = bass.AP(tensor=xT.tensor, offset=kc * P * T + w * WN,
                                  ap=[[T, P], [P * T, 2], [1, WN]])
                    nc.sync.dma_start(out=xw[:, kc:kc + 2, :], in_=src)
                if w == 0:
                    # deferred: not needed before the first attention chunk
                    nc.scalar.dma_start(out=wo_s, in_=wo)
                    nc.scalar.dma_start(out=mask_s, in_=mask)
                    nc.scalar.dma_start(out=ident_s, in_=ident)
                uid_q[w] = add_unit(q_unit(w, xw))
                for st in range(WN // P):
                    gt = (w * WN) // P + st
                    uid_k[gt] = add_unit(k_unit(w, st, xw))
                    uid_v[gt] = add_unit(v_unit(w, st, xw))
                state["enqueued"] += 1

        # ---- per-query-tile tail: normalize + transpose + out-proj ----
        def tail1_unit(b, icn, qs):
            acc = accA if qs < 2 else accB
            i2 = qs % 2
            gq0 = b * S + (icn * QT + qs) * P
            rcp = rcp_sb.tile([P, 2, 1], F32, name="rcp", tag="rcp")
            nc.vector.reciprocal(rcp, acc[:, i2, :, 64:65])
            nrmd = nrm_sb.tile([P, P], BF16, name="nrmd", tag="nrmd")
            # one op normalizes both heads: per-head reciprocal broadcast
            # across the 64 columns via a stride-0 inner dim
            rb = bass.AP(tensor=rcp.tensor, offset=rcp.offset,
                         ap=[rcp.ap[0], [1, 2], [0, 64]])
            nc.vector.tensor_mul(
                nrmd.rearrange("p (two xx) -> p two xx", two=2),
                acc[:, i2, :, 0:64], rb)
            yield 0.0
            tT = sm_ps.tile([P, P], BF16, name="tT", tag="sm")
            nc.tensor.transpose(tT, nrmd, ident_s)
            nc.vector.tensor_copy(cT[:, gq0:gq0 + P], tT)
            yield P * 0.42

        def tail2_unit(b, icn, qs, split_evac=False):
            gq0 = b * S + (icn * QT + qs) * P
            ob = ob_sb.tile([P, D], BF16, name="ob", tag="ob")
            for nn in range(D // 512):
                op = sm_ps.tile([P, 512], F32, name="op", tag="sm")
                nc.tensor.matmul(op, cT[:, gq0:gq0 + P],
                                 wo_s[:, nn * 512:(nn + 1) * 512],
                                 start=True, stop=True)
                if split_evac and nn % 2 == 0:
                    # ACT is idle at the kernel tail: split the evacuation
                    nc.scalar.activation(ob[:, nn * 512:(nn + 1) * 512], op,
                                         AF.Copy)
                else:
                    nc.vector.tensor_copy(ob[:, nn * 512:(nn + 1) * 512], op)
                yield 512 * 0.42
            # out-stores go via the idle Pool engine's SWDGE queue: their
            # data-ready waits must not block the in-order SP/HWDGE queue
            # that feeds latency-critical xw loads. Exception: the last
            # chunk's stores use the SP queue (empty by then) — the final
            # drain barrier otherwise waits ~1us of SWDGE descriptor
            # generation per store, serial on Pool
            if split_evac:
                nc.sync.dma_start(out=out[gq0:gq0 + P, :], in_=ob)
            else:
                nc.gpsimd.dma_start(out=out[gq0:gq0 + P, :], in_=ob)

        # ---- attention chunks ----
        def emit_ctx(b, icn, jt, ex):
            d = jt - icn * QT
            need_until(uid_v[b * NJ + jt])
            if jt == 0:
                # new chunk touches every acc region: prior chunks' tail
                # reads must already be emitted (WAR via program order);
                # a little filler covers the DVE normalize latency so the
                # first ctx doesn't stall on the acc WAR
                flush_tails()
                for _ in range(BFILL):
                    if work:
                        pump_one()
            for h in range(2):
                for qs in range(max(d, 0), QT):
                    acc = accA if qs < 2 else accB
                    # one start=True per acc bank per chunk: start pends the
                    # whole 2KB zero region, so the other groups' first
                    # accumulate materializes zero+write (lazy per-byte).
                    # All writers are PE matmuls -> program order holds.
                    nc.tensor.matmul(
                        acc[:, qs % 2, h, :],
                        ex[:, h, qs * P:(qs + 1) * P],
                        vA[:, b * NJ + jt, h * 65:(h + 1) * 65],
                        start=(jt == 0 and h == 0 and qs % 2 == 0),
                        stop=(jt == icn * QT + qs),
                        skip_group_check=True)
                    est["pe"] += 65 * 0.42
            if d >= 0:
                last = (b == B - 1 and icn == NIC - 1)
                tails.append((est["iter"] + TAIL_LAG, tail1_unit(b, icn, d)))
                tails2.append((est["iter"] + T2_LAG,
                               tail2_unit(b, icn, d, split_evac=last)))

        # lag-2 software pipeline: ctx(jt-2) is emitted after scores(jt), so
        # PE never reaches a ctx matmul before its exp (ACT, ~1.07us after
        # its scores) has retired; pend carries across chunk boundaries
        # p-state warmup: the PE ramps to full clock only after 3us of
        # continuous execution; burn the initial xw-DMA wait on dummy
        # matmuls so the first projections run at full speed
        for i in range(int(os.environ.get('KV2_WARM', '4'))):
            wp = sm_ps.tile([P, 512], F32, name="wp", tag="sm")
            nc.tensor.matmul(wp, warm[:, 0:P], warm, start=True, stop=True)

        pend = deque()
        for b in range(B):
            for icn in range(NIC):
                # per-chunk meter reset: a cumulative surplus from the
                # PE-bound early phase must not block pre-draining filler
                # during ACT-bound chunks
                est["pe"] = est["act"] = 0.0
                gi0 = b * S + icn * IC
                njt = (icn + 1) * QT
                nw = (b * S + (icn + 1) * IC + WN - 1) // WN
                enqueue_window(nw + AHEAD)
                need_until(uid_q[(b * S + icn * IC) // WN])
                last_chunk = (b == B - 1 and icn == NIC - 1)
                for jt in range(njt):
                    est["iter"] += 1
                    if icn >= 5:
                        # ACT-bound chunk: filler first, so PE doesn't sit
                        # stalled on the sc-buffer WAR (exp jt-2) and reset
                        # its p-state ramp
                        pump_to_balance()
                    # scores+exp go out first each iteration so the ACT
                    # engine's exp queue never waits behind tail/filler work
                    need_until(uid_k[b * NJ + jt])
                    il0 = max(0, jt * P - icn * IC)
                    gj0 = b * S + jt * P
                    sc = sc_ps.tile([P, 2, IC], F32, name="sc", tag="sc")
                    for h in range(2):
                        hp = h * 64
                        nc.tensor.matmul(
                            sc[:, h, il0:IC],
                            kT[hp:hp + 64, gj0:gj0 + P],
                            qT[hp:hp + 64, gi0 + il0:gi0 + IC],
                            start=True, stop=True)
                    est["pe"] += 2 * (IC - il0) * 0.42
                    ex = ex_sb.tile([P, 2, IC], BF16, name="ex", tag="ex")
                    nc.scalar.activation(ex[:, :, il0:IC], sc[:, :, il0:IC],
                                         AF.Exp, scale=0.125)
                    est["act"] += 2 * (IC - il0) * 0.84 + 185
                    if jt * P >= icn * IC:  # diagonal tile
                        # one strided op masks both heads: the mask operand
                        # repeats via a stride-0 middle dim
                        mb2 = bass.AP(tensor=mask_s.tensor,
                                      offset=mask_s.offset,
                                      ap=[mask_s.ap[0], [0, 2],
                                          mask_s.ap[-1]])
                        nc.vector.tensor_mul(ex[:, :, il0:il0 + P],
                                             ex[:, :, il0:il0 + P], mb2)
                    if last_chunk:
                        # no later chunk will absorb the backlog: drain all
                        # ready tails now so the end tail stays short
                        drain_tails(4)
                        if jt >= njt - 8:
                            flush_tails()
                            _drain(tails2, len(tails2), True)
                    else:
                        drain_tails(NTAILS)
                    if jt == njt - 2 and not last_chunk:
                        # flush the next chunk's q-window now, while PE is
                        # hot (full p-state) and the diag region's ACT load
                        # is light — the same flush after the boundary stall
                        # would run at half clock
                        b2, icn2 = (b, icn + 1) if icn + 1 < NIC else (b + 1, 0)
                        need_until(uid_q[(b2 * S + icn2 * IC) // WN])
                        need_until(uid_k[b2 * NJ])
                        need_until(uid_v[b2 * NJ])
                    pend.append((b, icn, jt, ex))
                    # deeper ctx lag in short early chunks: gives the prev
                    # chunk's last tail1 normalize more slack before the
                    # first ctx of this chunk hits the acc WAR
                    plag = 3 if icn <= int(os.environ.get('KV2_PLAGC', '7')) else 2
                    if len(pend) > plag:
                        emit_ctx(*pend.popleft())
                    pump_to_balance()
                # drain the ctx pipeline at chunk end so the last tiles'
                # tail1 normalizes get a couple of iterations of slack
                # before the next chunk's first ctx (acc WAR); filler first
                # so the last ctx doesn't stall on its just-emitted exp
                for _ in range(BFILL):
                    if work:
                        pump_one()
                while pend:
                    emit_ctx(*pend.popleft())
        flush_tails()
        _drain(tails2, len(tails2), True)
        while work:
            pump_one()

    nc.compile()
    return nc


def _warrange(w, bf16):
    # [D, CW] -> [P, D//P, CW] contiguous (the SBUF layout, so the DMA is
    # a single contiguous copy instead of 256B strided pieces)
    D, CW_ = w.shape
    return np.ascontiguousarray(
        w.reshape(D // P, P, CW_).transpose(1, 0, 2)).astype(bf16)


def make_in_maps(x, Wq, Wk, Wv, Wo):
    import ml_dtypes
    bf16 = ml_dtypes.bfloat16
    B, S, D = x.shape
    xT = np.ascontiguousarray(x.reshape(B * S, D).T).astype(bf16)
    mask = np.triu(np.ones((P, P), dtype=bf16))
    ident = np.eye(P, dtype=bf16)
    in_maps = []
    for c in range(N_CORES):
        cs = slice(c * CW, (c + 1) * CW)
        in_maps.append({
            "xT": xT,
            "wq": _warrange(Wq[:, cs], bf16),
            "wk": _warrange(Wk[:, cs], bf16),
            "wv": _warrange(Wv[:, cs], bf16),
            "wo": np.ascontiguousarray(Wo[cs, :]).astype(bf16),
            "mask": mask,
            "ident": ident,
        })
    return in_maps


_CACHED_NC = None


def kernel(x, Wq, Wk, Wv, Wo, bo, _trace=False):
    global _CACHED_NC
    x = np.asarray(x, dtype=np.float32)
    B, S, D = x.shape
    if _CACHED_NC is None:
        _CACHED_NC = build_program(S=S, B=B, D=D)
    nc = _CACHED_NC
    in_maps = make_in_maps(x, np.asarray(Wq), np.asarray(Wk),
                           np.asarray(Wv), np.asarray(Wo))
    res = None
    for attempt in range(3):
        try:
            res = run_bass_kernel_spmd(nc, in_maps, list(range(N_CORES)),
                                       trace=_trace)
            break
        except Exception:
            if attempt == 2:
                raise
    out = np.zeros((B * S, D), dtype=np.float32)
    for c in range(N_CORES):
        out += res.results[c]["out"].astype(np.float32)
    out += np.asarray(bo, dtype=np.float32)[None, :]
    if _trace:
        kernel._last_result = res
    return out.reshape(B, S, D)
